# revision 9
# baseline (speedup 1.0000x reference)
"""BiAttentionEncoder Trainium2 kernel (8 NeuronCores, data-parallel over batch).

Strategy (sharding_hint: data-parallel over batch):
  - 8 cores x 8 batch elements each; one SPMD program, per-core input shards.
  - Per core: embedding gather (indirect DMA) -> input projections (PE) ->
    bidirectional GRU recurrences (f+b stacked in PSUM partitions, freeze via
    z-gate mask trick) -> cross bilinear attention in d-major (transposed)
    layout -> tanh projections -> outputs. Host concatenates core outputs.

GRU per-step layout: PSUM tile [48, 512] rows = {f-r, b-r, f-z, b-z, f-n, b-n}
(8 rows each, batch in partitions); gates computed on [32,512]/[16,512] slices;
carry transposed each step via 4 PE transposes into the next step's lhsT.
Bias folding: bih + bhh for r/z folded into the phase-0 projection; bhh_n enters
the n-gate PSUM via a K=1 ones-row matmul (it must be scaled by r, so it cannot
be folded into xp).
"""

import sys
import time

sys.path.insert(0, "/opt/trn_rl_repo")

import numpy as np

import concourse.bass as bass
import concourse.tile as tile
from concourse import bacc, mybir
from concourse.bass import IndirectOffsetOnAxis
from concourse.bass_utils import run_bass_kernel_spmd
from concourse.masks import make_identity

F32 = mybir.dt.float32
I32 = mybir.dt.int32
ALU = mybir.AluOpType
AF = mybir.ActivationFunctionType
AX = mybir.AxisListType

V, E, H = 30000, 300, 512
B, ST, SN = 64, 128, 512
NCORES = 8
BPC = B // NCORES  # 8 batch elements per core
D2 = 2 * H  # 1024
STREAMS = ("t", "n")
TLEN = {"t": ST, "n": SN}


# ---------------------------------------------------------------- device build


def _gather_and_xp(nc, tc, pools, stream, emb_ap, idx_ap, wih_tiles, xp_ap, identity):
    """Phase 0 for one stream: gather emb rows, transpose, project to xp.

    xp_ap: DRAM [T, 48, 512]; row layout g*16 + dir*8 + b.
    """
    T = TLEN[stream]
    sb, ps = pools["sb0"], pools["ps0"]
    for b in range(BPC):
        for tc0 in range(T // 128):
            t0 = tc0 * 128
            idx_sb = sb.tile([128, 1], I32, tag="idx")
            nc.sync.dma_start(idx_sb[:, :], idx_ap[b, t0 : t0 + 128].rearrange("(a b) -> a b", b=1))
            emb_sb = sb.tile([128, E], F32, tag="embg")
            nc.gpsimd.indirect_dma_start(
                out=emb_sb[:, :],
                out_offset=None,
                in_=emb_ap[:, :],
                in_offset=IndirectOffsetOnAxis(ap=idx_sb[:, :1], axis=0),
            )
            # transpose to [E(+1 ones row), 128] chunks: 128, 128, 44(+ones)
            embT = []
            for k, (e0, ke) in enumerate(((0, 128), (128, 128), (256, 44))):
                ptr = ps.tile([128, 128], F32, tag="p0tr")
                nc.tensor.transpose(
                    ptr[:ke, :], emb_sb[:, e0 : e0 + ke], identity[:, :]
                )
                kk = ke if k < 2 else ke + 1
                et = sb.tile([kk, 128], F32, tag=f"embT{k}")
                if k == 2:
                    nc.vector.memset(et[:, :], 1.0)  # row 44 stays 1 (bias row)
                nc.vector.tensor_copy(et[:ke, :], ptr[:ke, :])
                embT.append(et)
            for d in range(2):  # 0=f, 1=b
                pxp = ps.tile([128, 3 * 512], F32, tag="p0xp")
                for g in range(3):
                    for k in range(3):
                        kk = embT[k].shape[0]
                        nc.tensor.matmul(
                            pxp[:, 512 * g : 512 * (g + 1)],
                            lhsT=embT[k][:, :],
                            rhs=wih_tiles[(stream, d)][k][:kk, 512 * g : 512 * (g + 1)],
                            start=(k == 0),
                            stop=(k == 2),
                        )
                # write [128, 3, 512] -> xp rows (g*16 + d*8 + b), times t0..t0+128
                xps = sb.tile([128, 3 * 512], F32, tag="xpsb")
                nc.scalar.copy(xps[:, :], pxp[:, :])
                dst = xp_ap[t0 : t0 + 128, d * 8 + b :: 16, :]
                nc.sync.dma_start(dst, xps[:, :].rearrange("p (g d) -> p g d", g=3))


def _gru_stream(nc, tc, pools, stream, xp_ap, whh_tiles, bhhn_f, bhhn_b,
                masks_m, masks_c, enc_ap, hidT_ap, identity, ones_row):
    """Bidirectional GRU over T steps; writes masked enc rows and final hT.

    PSUM col-position constraint: M=8 blocks may only start at partitions
    {0,32,64,96}. Layout: ghrz rows f-r@0, b-r@32, f-z@64, b-z@96;
    ghn rows f-n@0, b-n@32. Rows in between hold junk and are processed
    harmlessly by the (free-dim-bound) elementwise ops. h/hT keep the same
    {f@0, b@32} row pattern; hT chunk k lives at cols [40k, 40k+40).
    """
    T = TLEN[stream]
    sb, ps = pools["sb1"], pools["ps1"]

    h_prev = sb.tile([40, H], F32, tag="h", bufs=2)
    nc.vector.memset(h_prev[:, :], 0.0)
    hT_prev = sb.tile([128, 160], F32, tag="hT", bufs=2)
    nc.vector.memset(hT_prev[:, :], 0.0)

    for s in range(T):
        t_f, t_b = s, T - 1 - s
        xpt = sb.tile([104, 512], F32, tag="xpt", bufs=3)
        xptn = sb.tile([40, 512], F32, tag="xptn", bufs=3)
        nc.sync.dma_start(xpt[0:8, :], xp_ap[t_f, 0:8, :])
        nc.sync.dma_start(xpt[32:40, :], xp_ap[t_b, 8:16, :])
        nc.sync.dma_start(xpt[64:72, :], xp_ap[t_f, 16:24, :])
        nc.sync.dma_start(xpt[96:104, :], xp_ap[t_b, 24:32, :])
        nc.sync.dma_start(xptn[0:8, :], xp_ap[t_f, 32:40, :])
        nc.sync.dma_start(xptn[32:40, :], xp_ap[t_b, 40:48, :])

        ghrz = ps.tile([128, 512], F32, tag="ghrz", bufs=2)
        ghn = ps.tile([40, 512], F32, tag="ghn", bufs=2)
        # r/z matmuls first so the rz-chain starts as early as possible
        for k in range(4):
            for d in range(2):
                lhs = hT_prev[:, 40 * k + 32 * d : 40 * k + 32 * d + 8]
                nc.tensor.matmul(ghrz[32 * d : 32 * d + 8, :], lhsT=lhs,
                                 rhs=whh_tiles[(stream, d)][k][:, 0:512],
                                 start=(k == 0), stop=(k == 3))
                nc.tensor.matmul(ghrz[64 + 32 * d : 64 + 32 * d + 8, :], lhsT=lhs,
                                 rhs=whh_tiles[(stream, d)][k][:, 512:1024],
                                 start=(k == 0), stop=(k == 3),
                                 tile_position=(0, 64 + 32 * d))
        # n-gate: bhh_n preload via ones-row matmul (start=True clears psum)
        for d, bt in ((0, bhhn_f), (1, bhhn_b)):
            nc.tensor.matmul(ghn[32 * d : 32 * d + 8, :], lhsT=ones_row[:, :8],
                             rhs=bt[:, :], start=True, stop=False)
        for k in range(4):
            for d in range(2):
                lhs = hT_prev[:, 40 * k + 32 * d : 40 * k + 32 * d + 8]
                nc.tensor.matmul(ghn[32 * d : 32 * d + 8, :], lhsT=lhs,
                                 rhs=whh_tiles[(stream, d)][k][:, 1024:1536],
                                 start=False, stop=(k == 3))

        rz = sb.tile([104, 512], F32, tag="rz")
        nc.vector.tensor_tensor(rz[:, :], ghrz[0:104, :], xpt[:, :], op=ALU.add)
        rzs = sb.tile([104, 512], F32, tag="rzs")
        nc.scalar.activation(rzs[:, :], rz[:, :], AF.Sigmoid)
        zm = sb.tile([40, 512], F32, tag="zm")
        nc.vector.tensor_scalar(
            zm[:, :], rzs[64:104, :], masks_m[:, s : s + 1],
            masks_c[:, s : s + 1], ALU.mult, ALU.add,
        )
        nh = sb.tile([40, 512], F32, tag="nh")
        nc.vector.tensor_tensor(nh[:, :], ghn[0:40, :], rzs[0:40, :], op=ALU.mult)
        npre = sb.tile([40, 512], F32, tag="npre")
        nc.vector.tensor_tensor(npre[:, :], nh[:, :], xptn[:, :], op=ALU.add)
        ngate = sb.tile([40, 512], F32, tag="ngate")
        nc.scalar.activation(ngate[:, :], npre[:, :], AF.Tanh)
        dd = sb.tile([40, 512], F32, tag="dd")
        nc.vector.tensor_tensor(dd[:, :], h_prev[:, :], ngate[:, :], op=ALU.subtract)
        ee = sb.tile([40, 512], F32, tag="ee")
        nc.vector.tensor_tensor(ee[:, :], zm[:, :], dd[:, :], op=ALU.mult)
        h_new = sb.tile([40, H], F32, tag="h", bufs=2)
        nc.vector.tensor_tensor(h_new[:, :], ngate[:, :], ee[:, :], op=ALU.add)
        wm = sb.tile([40, 512], F32, tag="wm")
        nc.gpsimd.tensor_scalar(
            wm[:, :], h_new[:, :], masks_m[:, s : s + 1], None, ALU.mult
        )
        nc.sync.dma_start(enc_ap[0:BPC, t_f, 0:H], wm[0:8, :])
        nc.sync.dma_start(enc_ap[0:BPC, t_b, H:D2], wm[32:40, :])

        trp = ps.tile([128, 160], F32, tag="trp", bufs=2)
        for j in range(4):
            nc.tensor.transpose(
                trp[:, 40 * j : 40 * (j + 1)],
                h_new[:, 128 * j : 128 * (j + 1)],
                identity[0:40, 0:40],
            )
        hT_new = sb.tile([128, 160], F32, tag="hT", bufs=2)
        nc.vector.tensor_copy(hT_new[:, :], trp[:, :])
        h_prev, hT_prev = h_new, hT_new

    nc.sync.dma_start(hidT_ap[:, :], hT_prev[:, :])


def _softmax_free(nc, sb, psc, n_free, tag):
    """softmax over free dim of PSUM tile psc [P, n_free] -> sbuf tile."""
    mxn = sb.tile([128, 1], F32, tag=f"mx{tag}")
    nc.vector.tensor_reduce(mxn[:, :], psc[:, :], axis=AX.X, op=ALU.max, negate=True)
    ssum = sb.tile([128, 1], F32, tag=f"ss{tag}")
    ae = sb.tile([128, n_free], F32, tag=f"ae{tag}")
    nc.scalar.activation(ae[:, :], psc[:, :], AF.Exp, bias=mxn[:, :],
                         accum_out=ssum[:, :])
    rinv = sb.tile([128, 1], F32, tag=f"ri{tag}")
    nc.vector.reciprocal(rinv[:, :], ssum[:, :])
    a = sb.tile([128, n_free], F32, tag=f"a{tag}")
    nc.vector.tensor_scalar(a[:, :], ae[:, :], rinv[:, :], None, ALU.mult)
    return a


def build_program():
    nc = bacc.Bacc("TRN2", target_bir_lowering=False, debug=False,
                   num_devices=NCORES)

    # ---- I/O ----
    emb = nc.dram_tensor("emb", [V, E], F32, kind="ExternalInput").ap()
    idx = {s: nc.dram_tensor(f"idx_{s}", [BPC, TLEN[s]], I32,
                             kind="ExternalInput").ap() for s in STREAMS}
    masks = {s: nc.dram_tensor(f"masks_{s}", [80, TLEN[s]], F32,
                               kind="ExternalInput").ap() for s in STREAMS}
    wih = {}
    whh = {}
    for s in STREAMS:
        for d in range(2):
            wih[(s, d)] = nc.dram_tensor(f"wih_{s}{d}", [301, 3 * H], F32,
                                         kind="ExternalInput").ap()
            whh[(s, d)] = nc.dram_tensor(f"whh_{s}{d}", [H, 3 * H], F32,
                                         kind="ExternalInput").ap()
    bhhn = nc.dram_tensor("bhhn", [2, 2, H], F32, kind="ExternalInput").ap()
    biatt = nc.dram_tensor("biatt", [D2, D2], F32, kind="ExternalInput").ap()
    biatt_b = nc.dram_tensor("biatt_b", [D2], F32, kind="ExternalInput").ap()
    wt_aug = nc.dram_tensor("wt_aug", [2 * D2 + 1, D2], F32,
                            kind="ExternalInput").ap()
    wn_aug = nc.dram_tensor("wn_aug", [2 * D2 + 1, D2], F32,
                            kind="ExternalInput").ap()
    comb_aug = nc.dram_tensor("comb_aug", [D2 + 1, H], F32,
                              kind="ExternalInput").ap()

    enc_out = nc.dram_tensor("enc_out", [BPC, ST + SN, D2], F32,
                             kind="ExternalOutput").ap()
    hid_out = nc.dram_tensor("hid_out", [2, BPC, H], F32,
                             kind="ExternalOutput").ap()

    # ---- internal DRAM ----
    xp = {s: nc.dram_tensor(f"xp_{s}", [TLEN[s], 48, 512], F32).ap()
          for s in STREAMS}
    enc = {s: nc.dram_tensor(f"enc_{s}", [BPC, TLEN[s], D2], F32).ap()
           for s in STREAMS}
    hidT = nc.dram_tensor("hidT", [2, 128, 160], F32).ap()
    encT = {s: nc.dram_tensor(f"encT_{s}", [BPC, 128, 8 * TLEN[s]], F32).ap()
            for s in STREAMS}  # cols = (dchunk j, t)
    rnT = nc.dram_tensor("rnT", [BPC, 128, 8 * ST], F32).ap()   # r_news^T
    rtT = nc.dram_tensor("rtT", [BPC, 128, 8 * SN], F32).ap()   # r_tweets^T

    with tile.TileContext(nc) as tc, tc.tile_pool(name="const", bufs=1) as const:
        identity = const.tile([128, 128], F32)
        make_identity(nc, identity[:, :])
        ones_row = const.tile([1, 128], F32)
        nc.vector.memset(ones_row[:, :], 1.0)

        # ================= phase 0: gather + input projections ==============
        with tc.tile_pool(name="sb0", bufs=2) as sb0, \
             tc.tile_pool(name="ps0", bufs=2, space="PSUM") as ps0:
            pools = {"sb0": sb0, "ps0": ps0}
            wih_tiles = {}
            for s in STREAMS:
                for d in range(2):
                    tiles = []
                    for k, (e0, ke) in enumerate(((0, 128), (128, 128), (256, 45))):
                        wt_ = sb0.tile([ke, 3 * H], F32, tag=f"wih{s}{d}{k}",
                                       bufs=1)
                        nc.sync.dma_start(wt_[:, :], wih[(s, d)][e0 : e0 + ke, :])
                        tiles.append(wt_)
                    wih_tiles[(s, d)] = tiles
            for s in STREAMS:
                _gather_and_xp(nc, tc, pools, s, emb, idx[s], wih_tiles, xp[s],
                               identity)
        tc.strict_bb_all_engine_barrier()

        # ================= phases 1-2: GRU recurrences ======================
        for si, s in enumerate(("n", "t")):  # news first (longest)
            with tc.tile_pool(name=f"sb1{s}", bufs=2) as sb1, \
                 tc.tile_pool(name=f"ps1{s}", bufs=2, space="PSUM") as ps1:
                whh_tiles = {}
                for d in range(2):
                    tiles = []
                    for k in range(4):
                        wt_ = sb1.tile([128, 3 * H], F32, tag=f"whh{s}{d}{k}",
                                       bufs=1)
                        nc.sync.dma_start(
                            wt_[:, :], whh[(s, d)][128 * k : 128 * (k + 1), :]
                        )
                        tiles.append(wt_)
                    whh_tiles[(s, d)] = tiles
                si_ = 0 if s == "t" else 1
                bhhn_f = sb1.tile([1, H], F32, tag="bhhnf", bufs=1)
                nc.sync.dma_start(bhhn_f[:, :],
                                  bhhn[si_, 0].rearrange("(a b) -> a b", a=1))
                bhhn_b = sb1.tile([1, H], F32, tag="bhhnb", bufs=1)
                nc.sync.dma_start(bhhn_b[:, :],
                                  bhhn[si_, 1].rearrange("(a b) -> a b", a=1))
                masks_m = sb1.tile([40, TLEN[s]], F32, tag="masksm", bufs=1)
                nc.sync.dma_start(masks_m[:, :], masks[s][0:40, :])
                masks_c = sb1.tile([40, TLEN[s]], F32, tag="masksc", bufs=1)
                nc.sync.dma_start(masks_c[:, :], masks[s][40:80, :])
                pools = {"sb1": sb1, "ps1": ps1}
                _gru_stream(nc, tc, pools, s, xp[s], whh_tiles, bhhn_f, bhhn_b,
                            masks_m, masks_c, enc[s],
                            hidT[1 if s == "n" else 0], identity, ones_row)
            tc.strict_bb_all_engine_barrier()

        # ================= hidden: comb projection ==========================
        with tc.tile_pool(name="sbc", bufs=1) as sbc, \
             tc.tile_pool(name="psc", bufs=1, space="PSUM") as psc:
            comb_sb = []
            for k in range(8):
                ct = sbc.tile([128, H], F32, tag=f"comb{k}")
                nc.sync.dma_start(ct[:, :], comb_aug[128 * k : 128 * (k + 1), :])
                comb_sb.append(ct)
            comb_bias = sbc.tile([1, H], F32, tag="combb")
            nc.sync.dma_start(comb_bias[:, :], comb_aug[D2 : D2 + 1, :])
            hidT_sb = {}
            for si in range(2):
                for j in range(4):
                    ht = sbc.tile([128, 16], F32, tag=f"hidT{si}{j}")
                    src = hidT[si].rearrange("p (j g r) -> p j g r", j=4, g=5)
                    nc.sync.dma_start(ht[:, :], src[:, j, 0::4, :])
                    hidT_sb[(si, j)] = ht
            ph = psc.tile([16, H], F32, tag="phid")
            nc.tensor.matmul(ph[:, :], lhsT=ones_row[:, :16],
                             rhs=comb_bias[:, :], start=True, stop=False)
            for si in range(2):  # 0=tweets dims 0-511, 1=news dims 512-1023
                for j in range(4):
                    nc.tensor.matmul(
                        ph[:, :],
                        lhsT=hidT_sb[(si, j)][:, :],
                        rhs=comb_sb[4 * si + j][:, :],
                        start=False,
                        stop=(si == 1 and j == 3),
                    )
            hid_sb = sbc.tile([16, H], F32, tag="hid")
            nc.vector.tensor_copy(hid_sb[:, :], ph[:, :])
            nc.sync.dma_start(hid_out.rearrange("a b d -> (a b) d"), hid_sb[:, :])
        tc.strict_bb_all_engine_barrier()

        # ================= phase 3A: biatt + scores + r ====================
        with tc.tile_pool(name="sba", bufs=1) as sba, \
             tc.tile_pool(name="psa", bufs=1, space="PSUM") as psa:
            biatt_sb = []
            for k in range(8):
                bt = sba.tile([128, D2], F32, tag=f"biatt{k}", bufs=1)
                nc.sync.dma_start(bt[:, :], biatt[128 * k : 128 * (k + 1), :])
                biatt_sb.append(bt)
            bb_sb = sba.tile([128, 8], F32, tag="biattb", bufs=1)
            # biatt_b as [128,1] per chunk: load as [8,128] rows -> transpose
            bbT_tmp = sba.tile([8, 128], F32, tag="bbtmp", bufs=1)
            nc.sync.dma_start(bbT_tmp[:, :], biatt_b.rearrange("(a b) -> a b", a=8))
            pbb = psa.tile([128, 8], F32, tag="pbb", bufs=1)
            nc.tensor.transpose(pbb[:, :], bbT_tmp[:, :], identity[0:8, 0:8])
            nc.vector.tensor_copy(bb_sb[:, :], pbb[:, :])

            for b in range(BPC):
                # load natural enc tiles
                et_nat = sba.tile([128, D2], F32, tag="etnat")
                nc.sync.dma_start(et_nat[:, :], enc["t"][b])
                en_nat = []
                for i in range(4):
                    t_ = sba.tile([128, D2], F32, tag=f"ennat{i}")
                    nc.sync.dma_start(t_[:, :], enc["n"][b, 128 * i : 128 * (i + 1), :])
                    en_nat.append(t_)
                # build encT via PE transposes
                eTt = sba.tile([128, 8 * 128], F32, tag="eTt")
                for j in range(8):
                    ptr = psa.tile([128, 128], F32, tag="patr")
                    nc.tensor.transpose(ptr[:, :], et_nat[:, 128 * j : 128 * (j + 1)],
                                        identity[:, :])
                    nc.vector.tensor_copy(eTt[:, 128 * j : 128 * (j + 1)], ptr[:, :])
                nc.sync.dma_start(encT["t"][b], eTt[:, :])
                eTn = []
                for j in range(8):
                    tj = sba.tile([128, SN], F32, tag=f"eTn{j}")
                    eTn.append(tj)
                for i in range(4):
                    for j in range(8):
                        ptr = psa.tile([128, 128], F32, tag="patr")
                        nc.tensor.transpose(
                            ptr[:, :], en_nat[i][:, 128 * j : 128 * (j + 1)],
                            identity[:, :])
                        nc.vector.tensor_copy(
                            eTn[j][:, 128 * i : 128 * (i + 1)], ptr[:, :])
                for j in range(8):
                    nc.sync.dma_start(
                        encT["n"][b, :, SN * j : SN * (j + 1)], eTn[j][:, :])

                # tnT [d2, n] = biatt_W @ encT_n + b
                tnT = []
                for j in range(8):
                    ptn = psa.tile([128, SN], F32, tag="ptn")
                    for k in range(8):
                        nc.tensor.matmul(
                            ptn[:, :],
                            lhsT=biatt_sb[k][:, 128 * j : 128 * (j + 1)],
                            rhs=eTn[k][:, :], start=(k == 0), stop=(k == 7))
                    tj = sba.tile([128, SN], F32, tag=f"tnT{j}")
                    nc.vector.tensor_scalar(
                        tj[:, :], ptn[:, :], bb_sb[:, j : j + 1], None, ALU.add)
                    tnT.append(tj)
                # ttT [d2, t]
                ttT = sba.tile([128, 8 * 128], F32, tag="ttT")
                for j in range(8):
                    ptt = psa.tile([128, 128], F32, tag="ptt")
                    for k in range(8):
                        nc.tensor.matmul(
                            ptt[:, :],
                            lhsT=biatt_sb[k][:, 128 * j : 128 * (j + 1)],
                            rhs=eTt[:, 128 * k : 128 * (k + 1)],
                            start=(k == 0), stop=(k == 7))
                    nc.vector.tensor_scalar(
                        ttT[:, 128 * j : 128 * (j + 1)], ptt[:, :],
                        bb_sb[:, j : j + 1], None, ALU.add)

                # scores_tn [t, n] + softmax + aT
                psc_tn = psa.tile([128, SN], F32, tag="psctn")
                for k in range(8):
                    nc.tensor.matmul(
                        psc_tn[:, :], lhsT=eTt[:, 128 * k : 128 * (k + 1)],
                        rhs=tnT[k][:, :], start=(k == 0), stop=(k == 7))
                a_tn = _softmax_free(nc, sba, psc_tn, SN, "tn")
                aTtn = sba.tile([128, SN], F32, tag="aTtn")  # [n, t] chunks
                for i in range(4):
                    ptr = psa.tile([128, 128], F32, tag="patr")
                    nc.tensor.transpose(ptr[:, :], a_tn[:, 128 * i : 128 * (i + 1)],
                                        identity[:, :])
                    nc.vector.tensor_copy(aTtn[:, 128 * i : 128 * (i + 1)], ptr[:, :])
                # r_newsT [d, t] = enc_n^T(nat lhsT) @ aT
                rn_sb = sba.tile([128, 8 * 128], F32, tag="rnsb")
                for j in range(8):
                    prn = psa.tile([128, 128], F32, tag="prn")
                    for i in range(4):
                        nc.tensor.matmul(
                            prn[:, :],
                            lhsT=en_nat[i][:, 128 * j : 128 * (j + 1)],
                            rhs=aTtn[:, 128 * i : 128 * (i + 1)],
                            start=(i == 0), stop=(i == 3))
                    nc.vector.tensor_copy(rn_sb[:, 128 * j : 128 * (j + 1)],
                                          prn[:, :])
                nc.sync.dma_start(rnT[b], rn_sb[:, :])

                # scores_nt [n, t] + softmax (4 chunks) + aT
                aTnt = sba.tile([128, SN], F32, tag="aTnt")  # [t, n]
                for i in range(4):
                    psc_nt = psa.tile([128, 128], F32, tag="pscnt")
                    for k in range(8):
                        nc.tensor.matmul(
                            psc_nt[:, :],
                            lhsT=eTn[k][:, 128 * i : 128 * (i + 1)],
                            rhs=ttT[:, 128 * k : 128 * (k + 1)],
                            start=(k == 0), stop=(k == 7))
                    a_i = _softmax_free(nc, sba, psc_nt, 128, "nt")
                    ptr = psa.tile([128, 128], F32, tag="patr")
                    nc.tensor.transpose(ptr[:, :], a_i[:, :], identity[:, :])
                    nc.vector.tensor_copy(aTnt[:, 128 * i : 128 * (i + 1)], ptr[:, :])
                # r_tweetsT [d, n] = enc_t(nat lhsT) @ aTnt
                rt_sb = sba.tile([128, 8 * SN], F32, tag="rtsb")
                for j in range(8):
                    prt = psa.tile([128, SN], F32, tag="prt")
                    nc.tensor.matmul(
                        prt[:, :], lhsT=et_nat[:, 128 * j : 128 * (j + 1)],
                        rhs=aTnt[:, :], start=True, stop=True)
                    nc.vector.tensor_copy(rt_sb[:, SN * j : SN * (j + 1)], prt[:, :])
                nc.sync.dma_start(rtT[b], rt_sb[:, :])
        tc.strict_bb_all_engine_barrier()

        # ================= phase 3B: v_t =====================================
        with tc.tile_pool(name="sbb", bufs=2) as sbb, \
             tc.tile_pool(name="psb", bufs=2, space="PSUM") as psb:
            wt_sb = []
            for k in range(16):
                wt_ = sbb.tile([128, D2], F32, tag=f"wt{k}", bufs=1)
                nc.sync.dma_start(wt_[:, :], wt_aug[128 * k : 128 * (k + 1), :])
                wt_sb.append(wt_)
            wt_bias = sbb.tile([1, D2], F32, tag="wtb", bufs=1)
            nc.sync.dma_start(wt_bias[:, :], wt_aug[2 * D2 : 2 * D2 + 1, :])
            for b in range(BPC):
                eTt = sbb.tile([128, 8 * 128], F32, tag="eTtb")
                nc.sync.dma_start(eTt[:, :], encT["t"][b])
                rn_sb = sbb.tile([128, 8 * 128], F32, tag="rnb")
                nc.sync.dma_start(rn_sb[:, :], rnT[b])
                pv = psb.tile([128, D2], F32, tag="pv")
                for n2 in range(2):
                    nsl = slice(512 * n2, 512 * (n2 + 1))
                    nc.tensor.matmul(pv[:, nsl], lhsT=ones_row[:, :],
                                     rhs=wt_bias[:, nsl], start=True, stop=False)
                    for k in range(16):
                        lhs = (eTt[:, 128 * k : 128 * (k + 1)] if k < 8
                               else rn_sb[:, 128 * (k - 8) : 128 * (k - 7)])
                        nc.tensor.matmul(pv[:, nsl], lhsT=lhs,
                                         rhs=wt_sb[k][:, nsl],
                                         start=False, stop=(k == 15))
                vt = sbb.tile([128, D2], F32, tag="vt")
                nc.scalar.activation(vt[:, :], pv[:, :], AF.Tanh)
                nc.sync.dma_start(enc_out[b, 0:ST, :], vt[:, :])
        tc.strict_bb_all_engine_barrier()

        # ================= phase 3C: v_n =====================================
        with tc.tile_pool(name="sbn", bufs=2) as sbn, \
             tc.tile_pool(name="psn", bufs=2, space="PSUM") as psn:
            wn_sb = []
            for k in range(16):
                wn_ = sbn.tile([128, D2], F32, tag=f"wn{k}", bufs=1)
                nc.sync.dma_start(wn_[:, :], wn_aug[128 * k : 128 * (k + 1), :])
                wn_sb.append(wn_)
            wn_bias = sbn.tile([1, D2], F32, tag="wnb", bufs=1)
            nc.sync.dma_start(wn_bias[:, :], wn_aug[2 * D2 : 2 * D2 + 1, :])
            for b in range(BPC):
                eTn = []
                for j in range(8):
                    tj = sbn.tile([128, SN], F32, tag=f"eTnc{j}")
                    nc.sync.dma_start(tj[:, :], encT["n"][b, :, SN * j : SN * (j + 1)])
                    eTn.append(tj)
                rt_sb = sbn.tile([128, 8 * SN], F32, tag="rtb")
                nc.sync.dma_start(rt_sb[:, :], rtT[b])
                for m in range(4):  # n chunks of 128
                    pv = psn.tile([128, D2], F32, tag="pvn")
                    for n2 in range(2):
                        nsl = slice(512 * n2, 512 * (n2 + 1))
                        nc.tensor.matmul(pv[:, nsl], lhsT=ones_row[:, :],
                                         rhs=wn_bias[:, nsl], start=True, stop=False)
                        for k in range(16):
                            lhs = (eTn[k][:, 128 * m : 128 * (m + 1)] if k < 8
                                   else rt_sb[:, SN * (k - 8) + 128 * m :
                                              SN * (k - 8) + 128 * (m + 1)])
                            nc.tensor.matmul(pv[:, nsl], lhsT=lhs,
                                             rhs=wn_sb[k][:, nsl],
                                             start=False, stop=(k == 15))
                    vn = sbn.tile([128, D2], F32, tag="vn")
                    nc.scalar.activation(vn[:, :], pv[:, :], AF.Tanh)
                    nc.sync.dma_start(
                        enc_out[b, ST + 128 * m : ST + 128 * (m + 1), :], vn[:, :])

    nc.compile()
    return nc


# ---------------------------------------------------------------- host side

_NC_CACHE = {}


def _get_program():
    if "nc" not in _NC_CACHE:
        t0 = time.time()
        _NC_CACHE["nc"] = build_program()
        print(f"[kernel] program build+compile: {time.time() - t0:.1f}s",
              file=sys.stderr)
    return _NC_CACHE["nc"]


def _prep_in_maps(inputs):
    f32 = lambda x: np.ascontiguousarray(np.asarray(x), dtype=np.float32)
    i32 = lambda x: np.ascontiguousarray(np.asarray(x), dtype=np.int32)

    tok = {"t": i32(inputs["input_tweets"]), "n": i32(inputs["input_news"])}
    for s in STREAMS:
        tok[s] = np.where(tok[s] > V, 3, tok[s]).astype(np.int32)
    lens = {"t": i32(inputs["lengths_tweets"]), "n": i32(inputs["lengths_news"])}

    emb = f32(inputs["emb_W"])
    shared = {"emb": emb, "biatt": f32(inputs["biatt_W"]).T.copy(),
              "biatt_b": f32(inputs["biatt_b"])}
    for s, pre in (("t", "gt"), ("n", "gn")):
        for d, dn in ((0, "f"), (1, "b")):
            Wih = f32(inputs[f"{pre}_Wih_{dn}"])
            Whh = f32(inputs[f"{pre}_Whh_{dn}"])
            bih = f32(inputs[f"{pre}_bih_{dn}"])
            bhh = f32(inputs[f"{pre}_bhh_{dn}"])
            bias = bih.copy()
            bias[: 2 * H] += bhh[: 2 * H]
            shared[f"wih_{s}{d}"] = np.ascontiguousarray(
                np.vstack([Wih.T, bias[None, :]]))
            shared[f"whh_{s}{d}"] = np.ascontiguousarray(Whh.T)
    bhhn = np.zeros((2, 2, H), np.float32)
    for si, (s, pre) in enumerate((("t", "gt"), ("n", "gn"))):
        for d, dn in ((0, "f"), (1, "b")):
            bhhn[si, d] = f32(inputs[f"{pre}_bhh_{dn}"])[2 * H :]
    shared["bhhn"] = bhhn
    shared["wt_aug"] = np.ascontiguousarray(
        np.vstack([f32(inputs["wt_W"]).T, f32(inputs["wt_b"])[None, :]]))
    shared["wn_aug"] = np.ascontiguousarray(
        np.vstack([f32(inputs["wn_W"]).T, f32(inputs["wn_b"])[None, :]]))
    shared["comb_aug"] = np.ascontiguousarray(
        np.vstack([f32(inputs["comb_W"]).T, f32(inputs["comb_b"])[None, :]]))

    in_maps = []
    for c in range(NCORES):
        bs = slice(c * BPC, (c + 1) * BPC)
        m = dict(shared)
        for s in STREAMS:
            T = TLEN[s]
            m[f"idx_{s}"] = np.ascontiguousarray(tok[s][bs])
            ln = lens[s][bs]
            t_ar = np.arange(T)
            mf = (t_ar[:, None] < ln[None, :]).astype(np.float32)      # [T, 8]
            mb = ((T - 1 - t_ar)[:, None] < ln[None, :]).astype(np.float32)
            marr = np.zeros((80, T), np.float32)
            marr[0:8] = mf.T
            marr[32:40] = mb.T
            marr[40:48] = 1.0 - mf.T
            marr[72:80] = 1.0 - mb.T
            m[f"masks_{s}"] = np.ascontiguousarray(marr)
        in_maps.append(m)
    return in_maps


def kernel(**inputs):
    nc = _get_program()
    in_maps = _prep_in_maps(inputs)
    res = run_bass_kernel_spmd(nc, in_maps, list(range(NCORES)))
    enc = np.concatenate([res.results[c]["enc_out"] for c in range(NCORES)], 0)
    hid = np.concatenate([res.results[c]["hid_out"] for c in range(NCORES)], 1)
    return enc.astype(np.float32), hid.astype(np.float32)


if __name__ == "__main__":
    inp = dict(np.load("/root/problem/inputs.npz"))
    t0 = time.time()
    out, hid = kernel(**inp)
    print("total wall:", time.time() - t0)
    exp = np.load("/root/problem/expected.npz")
    for name, got, ex in (("enc", out, exp["out"]), ("hid", hid, exp["hid"])):
        err = np.abs(got - ex).max()
        print(f"{name}: absmax_err={err:.3e} scale={np.abs(ex).max():.3f}")


# revision 10
# speedup vs baseline: 1.6315x; 1.6315x over previous
"""BiAttentionEncoder Trainium2 kernel (8 NeuronCores, data-parallel over batch).

Strategy (sharding_hint: data-parallel over batch):
  - 8 cores x 8 batch elements each; one SPMD program, per-core input shards.
  - Per core: embedding gather (indirect DMA) -> input projections (PE) ->
    bidirectional GRU recurrences (f+b stacked in PSUM partitions, freeze via
    z-gate mask trick) -> cross bilinear attention in d-major (transposed)
    layout -> tanh projections -> outputs. Host concatenates core outputs.

GRU per-step layout: PSUM tile [48, 512] rows = {f-r, b-r, f-z, b-z, f-n, b-n}
(8 rows each, batch in partitions); gates computed on [32,512]/[16,512] slices;
carry transposed each step via 4 PE transposes into the next step's lhsT.
Bias folding: bih + bhh for r/z folded into the phase-0 projection; bhh_n enters
the n-gate PSUM via a K=1 ones-row matmul (it must be scaled by r, so it cannot
be folded into xp).
"""

import os
import sys
import time

sys.path.insert(0, "/opt/trn_rl_repo")

import numpy as np

import concourse.bass as bass
import concourse.tile as tile
from concourse import bacc, mybir
from concourse.bass import IndirectOffsetOnAxis
from concourse.bass_utils import run_bass_kernel_spmd
from concourse.masks import make_identity

F32 = mybir.dt.float32
I32 = mybir.dt.int32
ALU = mybir.AluOpType
AF = mybir.ActivationFunctionType
AX = mybir.AxisListType

V, E, H = 30000, 300, 512
B, ST, SN = 64, 128, 512
NCORES = 8
BPC = B // NCORES  # 8 batch elements per core
D2 = 2 * H  # 1024
STREAMS = ("t", "n")
TLEN = {"t": ST, "n": SN}


# ---------------------------------------------------------------- device build


def _gather_and_xp(nc, tc, pools, stream, emb_ap, idx_ap, wih_tiles, xp_ap, identity):
    """Phase 0 for one stream: gather emb rows, transpose, project to xp.

    xp_ap: DRAM [T, 48, 512]; row layout g*16 + dir*8 + b.
    """
    T = TLEN[stream]
    sb, ps = pools["sb0"], pools["ps0"]
    for b in range(BPC):
        for tc0 in range(T // 128):
            t0 = tc0 * 128
            idx_sb = sb.tile([128, 1], I32, tag="idx")
            nc.sync.dma_start(idx_sb[:, :], idx_ap[b, t0 : t0 + 128].rearrange("(a b) -> a b", b=1))
            emb_sb = sb.tile([128, E], F32, tag="embg")
            nc.gpsimd.indirect_dma_start(
                out=emb_sb[:, :],
                out_offset=None,
                in_=emb_ap[:, :],
                in_offset=IndirectOffsetOnAxis(ap=idx_sb[:, :1], axis=0),
            )
            # transpose to [E(+1 ones row), 128] chunks: 128, 128, 44(+ones)
            embT = []
            for k, (e0, ke) in enumerate(((0, 128), (128, 128), (256, 44))):
                ptr = ps.tile([128, 128], F32, tag="p0tr")
                nc.tensor.transpose(
                    ptr[:ke, :], emb_sb[:, e0 : e0 + ke], identity[:, :]
                )
                kk = ke if k < 2 else ke + 1
                et = sb.tile([kk, 128], F32, tag=f"embT{k}")
                if k == 2:
                    nc.vector.memset(et[:, :], 1.0)  # row 44 stays 1 (bias row)
                nc.vector.tensor_copy(et[:ke, :], ptr[:ke, :])
                embT.append(et)
            for d in range(2):  # 0=f, 1=b
                pxp = ps.tile([128, 3 * 512], F32, tag="p0xp")
                for g in range(3):
                    for k in range(3):
                        kk = embT[k].shape[0]
                        nc.tensor.matmul(
                            pxp[:, 512 * g : 512 * (g + 1)],
                            lhsT=embT[k][:, :],
                            rhs=wih_tiles[(stream, d)][k][:kk, 512 * g : 512 * (g + 1)],
                            start=(k == 0),
                            stop=(k == 2),
                        )
                # write [128, 3, 512] -> xp rows (g*16 + d*8 + b), times t0..t0+128
                xps = sb.tile([128, 3 * 512], F32, tag="xpsb")
                nc.scalar.copy(xps[:, :], pxp[:, :])
                dst = xp_ap[t0 : t0 + 128, d * 8 + b :: 16, :]
                nc.sync.dma_start(dst, xps[:, :].rearrange("p (g d) -> p g d", g=3))


def _gru_stream(nc, tc, pools, stream, xp_ap, whh_tiles, bhhn_f, bhhn_b,
                masks_m, masks_c, enc_ap, hidT_ap, identity, ones_row):
    """Bidirectional GRU over T steps; writes masked enc rows and final hT.

    PSUM col-position constraint: M=8 blocks may only start at partitions
    {0,32,64,96}. Layout: ghrz rows f-r@0, b-r@32, f-z@64, b-z@96;
    ghn rows f-n@0, b-n@32. Rows in between hold junk and are processed
    harmlessly by the (free-dim-bound) elementwise ops. h/hT keep the same
    {f@0, b@32} row pattern; hT chunk k lives at cols [40k, 40k+40).
    """
    T = TLEN[stream]
    if os.environ.get("KERNEL_SKIP_GRU"):
        T = 4  # timing-isolation mode: run only 4 steps per stream
    sb, ps = pools["sb1"], pools["ps1"]

    h_prev = sb.tile([40, H], F32, tag="h", bufs=2)
    nc.vector.memset(h_prev[:, :], 0.0)
    hT_prev = sb.tile([128, 160], F32, tag="hT", bufs=2)
    nc.vector.memset(hT_prev[:, :], 0.0)

    for s in range(T):
        t_f, t_b = s, TLEN[stream] - 1 - s
        xpt = sb.tile([104, 512], F32, tag="xpt", bufs=3)
        xptn = sb.tile([40, 512], F32, tag="xptn", bufs=3)
        nc.sync.dma_start(xpt[0:8, :], xp_ap[t_f, 0:8, :])
        nc.sync.dma_start(xpt[32:40, :], xp_ap[t_b, 8:16, :])
        nc.sync.dma_start(xpt[64:72, :], xp_ap[t_f, 16:24, :])
        nc.sync.dma_start(xpt[96:104, :], xp_ap[t_b, 24:32, :])
        nc.sync.dma_start(xptn[0:8, :], xp_ap[t_f, 32:40, :])
        nc.sync.dma_start(xptn[32:40, :], xp_ap[t_b, 40:48, :])

        ghrz = ps.tile([128, 512], F32, tag="ghrz", bufs=2)
        ghn = ps.tile([40, 512], F32, tag="ghn", bufs=2)
        # r/z matmuls first so the rz-chain starts as early as possible
        for k in range(4):
            for d in range(2):
                lhs = hT_prev[:, 40 * k + 32 * d : 40 * k + 32 * d + 8]
                nc.tensor.matmul(ghrz[32 * d : 32 * d + 8, :], lhsT=lhs,
                                 rhs=whh_tiles[(stream, d)][k][:, 0:512],
                                 start=(k == 0), stop=(k == 3))
                nc.tensor.matmul(ghrz[64 + 32 * d : 64 + 32 * d + 8, :], lhsT=lhs,
                                 rhs=whh_tiles[(stream, d)][k][:, 512:1024],
                                 start=(k == 0), stop=(k == 3),
                                 tile_position=(0, 64 + 32 * d))
        # n-gate: bhh_n preload via ones-row matmul (start=True clears psum)
        for d, bt in ((0, bhhn_f), (1, bhhn_b)):
            nc.tensor.matmul(ghn[32 * d : 32 * d + 8, :], lhsT=ones_row[:, :8],
                             rhs=bt[:, :], start=True, stop=False)
        for k in range(4):
            for d in range(2):
                lhs = hT_prev[:, 40 * k + 32 * d : 40 * k + 32 * d + 8]
                nc.tensor.matmul(ghn[32 * d : 32 * d + 8, :], lhsT=lhs,
                                 rhs=whh_tiles[(stream, d)][k][:, 1024:1536],
                                 start=False, stop=(k == 3))

        rz = sb.tile([104, 512], F32, tag="rz")
        nc.vector.tensor_tensor(rz[:, :], ghrz[0:104, :], xpt[:, :], op=ALU.add)
        rzs = sb.tile([104, 512], F32, tag="rzs")
        nc.scalar.activation(rzs[:, :], rz[:, :], AF.Sigmoid)
        zm = sb.tile([40, 512], F32, tag="zm")
        nc.vector.tensor_scalar(
            zm[:, :], rzs[64:104, :], masks_m[:, s : s + 1],
            masks_c[:, s : s + 1], ALU.mult, ALU.add,
        )
        nh = sb.tile([40, 512], F32, tag="nh")
        nc.vector.tensor_tensor(nh[:, :], ghn[0:40, :], rzs[0:40, :], op=ALU.mult)
        npre = sb.tile([40, 512], F32, tag="npre")
        nc.vector.tensor_tensor(npre[:, :], nh[:, :], xptn[:, :], op=ALU.add)
        ngate = sb.tile([40, 512], F32, tag="ngate")
        nc.scalar.activation(ngate[:, :], npre[:, :], AF.Tanh)
        dd = sb.tile([40, 512], F32, tag="dd")
        nc.vector.tensor_tensor(dd[:, :], h_prev[:, :], ngate[:, :], op=ALU.subtract)
        ee = sb.tile([40, 512], F32, tag="ee")
        nc.vector.tensor_tensor(ee[:, :], zm[:, :], dd[:, :], op=ALU.mult)
        h_new = sb.tile([40, H], F32, tag="h", bufs=2)
        nc.vector.tensor_tensor(h_new[:, :], ngate[:, :], ee[:, :], op=ALU.add)
        wm = sb.tile([40, 512], F32, tag="wm")
        nc.gpsimd.tensor_scalar(
            wm[:, :], h_new[:, :], masks_m[:, s : s + 1], None, ALU.mult
        )
        nc.sync.dma_start(enc_ap[0:BPC, t_f, 0:H], wm[0:8, :])
        nc.sync.dma_start(enc_ap[0:BPC, t_b, H:D2], wm[32:40, :])

        trp = ps.tile([128, 160], F32, tag="trp", bufs=2)
        for j in range(4):
            nc.tensor.transpose(
                trp[:, 40 * j : 40 * (j + 1)],
                h_new[:, 128 * j : 128 * (j + 1)],
                identity[0:40, 0:40],
            )
        hT_new = sb.tile([128, 160], F32, tag="hT", bufs=2)
        nc.vector.tensor_copy(hT_new[:, :], trp[:, :])
        h_prev, hT_prev = h_new, hT_new

    nc.sync.dma_start(hidT_ap[:, :], hT_prev[:, :])


def _softmax_free(nc, sb, psc, n_free, tag):
    """softmax over free dim of PSUM tile psc [P, n_free] -> sbuf tile."""
    mxn = sb.tile([128, 1], F32, tag=f"mx{tag}")
    nc.vector.tensor_reduce(mxn[:, :], psc[:, :], axis=AX.X, op=ALU.max, negate=True)
    ssum = sb.tile([128, 1], F32, tag=f"ss{tag}")
    ae = sb.tile([128, n_free], F32, tag=f"ae{tag}")
    nc.scalar.activation(ae[:, :], psc[:, :], AF.Exp, bias=mxn[:, :],
                         accum_out=ssum[:, :])
    rinv = sb.tile([128, 1], F32, tag=f"ri{tag}")
    nc.vector.reciprocal(rinv[:, :], ssum[:, :])
    a = sb.tile([128, n_free], F32, tag=f"a{tag}")
    nc.vector.tensor_scalar(a[:, :], ae[:, :], rinv[:, :], None, ALU.mult)
    return a


def build_program():
    nc = bacc.Bacc("TRN2", target_bir_lowering=False, debug=False,
                   num_devices=NCORES)

    # ---- I/O ----
    emb = nc.dram_tensor("emb", [V, E], F32, kind="ExternalInput").ap()
    idx = {s: nc.dram_tensor(f"idx_{s}", [BPC, TLEN[s]], I32,
                             kind="ExternalInput").ap() for s in STREAMS}
    masks = {s: nc.dram_tensor(f"masks_{s}", [80, TLEN[s]], F32,
                               kind="ExternalInput").ap() for s in STREAMS}
    wih = {}
    whh = {}
    for s in STREAMS:
        for d in range(2):
            wih[(s, d)] = nc.dram_tensor(f"wih_{s}{d}", [301, 3 * H], F32,
                                         kind="ExternalInput").ap()
            whh[(s, d)] = nc.dram_tensor(f"whh_{s}{d}", [H, 3 * H], F32,
                                         kind="ExternalInput").ap()
    bhhn = nc.dram_tensor("bhhn", [2, 2, H], F32, kind="ExternalInput").ap()
    biatt = nc.dram_tensor("biatt", [D2, D2], F32, kind="ExternalInput").ap()
    biatt_b = nc.dram_tensor("biatt_b", [D2], F32, kind="ExternalInput").ap()
    wt_aug = nc.dram_tensor("wt_aug", [2 * D2 + 1, D2], F32,
                            kind="ExternalInput").ap()
    wn_aug = nc.dram_tensor("wn_aug", [2 * D2 + 1, D2], F32,
                            kind="ExternalInput").ap()
    comb_aug = nc.dram_tensor("comb_aug", [D2 + 1, H], F32,
                              kind="ExternalInput").ap()

    enc_out = nc.dram_tensor("enc_out", [BPC, ST + SN, D2], F32,
                             kind="ExternalOutput").ap()
    hid_out = nc.dram_tensor("hid_out", [2, BPC, H], F32,
                             kind="ExternalOutput").ap()

    # ---- internal DRAM ----
    xp = {s: nc.dram_tensor(f"xp_{s}", [TLEN[s], 48, 512], F32).ap()
          for s in STREAMS}
    enc = {s: nc.dram_tensor(f"enc_{s}", [BPC, TLEN[s], D2], F32).ap()
           for s in STREAMS}
    hidT = nc.dram_tensor("hidT", [2, 128, 160], F32).ap()
    encT = {s: nc.dram_tensor(f"encT_{s}", [BPC, 128, 8 * TLEN[s]], F32).ap()
            for s in STREAMS}  # cols = (dchunk j, t)
    rnT = nc.dram_tensor("rnT", [BPC, 128, 8 * ST], F32).ap()   # r_news^T
    rtT = nc.dram_tensor("rtT", [BPC, 128, 8 * SN], F32).ap()   # r_tweets^T

    with tile.TileContext(nc) as tc, tc.tile_pool(name="const", bufs=1) as const:
        identity = const.tile([128, 128], F32)
        make_identity(nc, identity[:, :])
        ones_row = const.tile([1, 128], F32)
        nc.vector.memset(ones_row[:, :], 1.0)

        # ================= phase 0: gather + input projections ==============
        with tc.tile_pool(name="sb0", bufs=2) as sb0, \
             tc.tile_pool(name="ps0", bufs=2, space="PSUM") as ps0:
            pools = {"sb0": sb0, "ps0": ps0}
            wih_tiles = {}
            for s in STREAMS:
                for d in range(2):
                    tiles = []
                    for k, (e0, ke) in enumerate(((0, 128), (128, 128), (256, 45))):
                        wt_ = sb0.tile([ke, 3 * H], F32, tag=f"wih{s}{d}{k}",
                                       bufs=1)
                        nc.sync.dma_start(wt_[:, :], wih[(s, d)][e0 : e0 + ke, :])
                        tiles.append(wt_)
                    wih_tiles[(s, d)] = tiles
            for s in STREAMS:
                _gather_and_xp(nc, tc, pools, s, emb, idx[s], wih_tiles, xp[s],
                               identity)
        tc.strict_bb_all_engine_barrier()

        # ================= phases 1-2: GRU recurrences ======================
        for si, s in enumerate(("n", "t")):  # news first (longest)
            with tc.tile_pool(name=f"sb1{s}", bufs=2) as sb1, \
                 tc.tile_pool(name=f"ps1{s}", bufs=2, space="PSUM") as ps1:
                whh_tiles = {}
                for d in range(2):
                    tiles = []
                    for k in range(4):
                        wt_ = sb1.tile([128, 3 * H], F32, tag=f"whh{s}{d}{k}",
                                       bufs=1)
                        nc.sync.dma_start(
                            wt_[:, :], whh[(s, d)][128 * k : 128 * (k + 1), :]
                        )
                        tiles.append(wt_)
                    whh_tiles[(s, d)] = tiles
                si_ = 0 if s == "t" else 1
                bhhn_f = sb1.tile([1, H], F32, tag="bhhnf", bufs=1)
                nc.sync.dma_start(bhhn_f[:, :],
                                  bhhn[si_, 0].rearrange("(a b) -> a b", a=1))
                bhhn_b = sb1.tile([1, H], F32, tag="bhhnb", bufs=1)
                nc.sync.dma_start(bhhn_b[:, :],
                                  bhhn[si_, 1].rearrange("(a b) -> a b", a=1))
                masks_m = sb1.tile([40, TLEN[s]], F32, tag="masksm", bufs=1)
                nc.sync.dma_start(masks_m[:, :], masks[s][0:40, :])
                masks_c = sb1.tile([40, TLEN[s]], F32, tag="masksc", bufs=1)
                nc.sync.dma_start(masks_c[:, :], masks[s][40:80, :])
                pools = {"sb1": sb1, "ps1": ps1}
                _gru_stream(nc, tc, pools, s, xp[s], whh_tiles, bhhn_f, bhhn_b,
                            masks_m, masks_c, enc[s],
                            hidT[1 if s == "n" else 0], identity, ones_row)
            tc.strict_bb_all_engine_barrier()

        # ================= hidden: comb projection ==========================
        with tc.tile_pool(name="sbc", bufs=1) as sbc, \
             tc.tile_pool(name="psc", bufs=1, space="PSUM") as psc:
            comb_sb = []
            for k in range(8):
                ct = sbc.tile([128, H], F32, tag=f"comb{k}")
                nc.sync.dma_start(ct[:, :], comb_aug[128 * k : 128 * (k + 1), :])
                comb_sb.append(ct)
            comb_bias = sbc.tile([1, H], F32, tag="combb")
            nc.sync.dma_start(comb_bias[:, :], comb_aug[D2 : D2 + 1, :])
            hidT_sb = {}
            for si in range(2):
                for j in range(4):
                    ht = sbc.tile([128, 16], F32, tag=f"hidT{si}{j}")
                    src = hidT[si].rearrange("p (j g r) -> p j g r", j=4, g=5)
                    nc.sync.dma_start(ht[:, :], src[:, j, 0::4, :])
                    hidT_sb[(si, j)] = ht
            ph = psc.tile([16, H], F32, tag="phid")
            nc.tensor.matmul(ph[:, :], lhsT=ones_row[:, :16],
                             rhs=comb_bias[:, :], start=True, stop=False)
            for si in range(2):  # 0=tweets dims 0-511, 1=news dims 512-1023
                for j in range(4):
                    nc.tensor.matmul(
                        ph[:, :],
                        lhsT=hidT_sb[(si, j)][:, :],
                        rhs=comb_sb[4 * si + j][:, :],
                        start=False,
                        stop=(si == 1 and j == 3),
                    )
            hid_sb = sbc.tile([16, H], F32, tag="hid")
            nc.vector.tensor_copy(hid_sb[:, :], ph[:, :])
            nc.sync.dma_start(hid_out.rearrange("a b d -> (a b) d"), hid_sb[:, :])
        tc.strict_bb_all_engine_barrier()

        # ================= phase 3A: biatt + scores + r ====================
        with tc.tile_pool(name="sba", bufs=1) as sba, \
             tc.tile_pool(name="psa", bufs=1, space="PSUM") as psa:
            biatt_sb = []
            for k in range(8):
                bt = sba.tile([128, D2], F32, tag=f"biatt{k}", bufs=1)
                nc.sync.dma_start(bt[:, :], biatt[128 * k : 128 * (k + 1), :])
                biatt_sb.append(bt)
            bb_sb = sba.tile([128, 8], F32, tag="biattb", bufs=1)
            # biatt_b as [128,1] per chunk: load as [8,128] rows -> transpose
            bbT_tmp = sba.tile([8, 128], F32, tag="bbtmp", bufs=1)
            nc.sync.dma_start(bbT_tmp[:, :], biatt_b.rearrange("(a b) -> a b", a=8))
            pbb = psa.tile([128, 8], F32, tag="pbb", bufs=1)
            nc.tensor.transpose(pbb[:, :], bbT_tmp[:, :], identity[0:8, 0:8])
            nc.vector.tensor_copy(bb_sb[:, :], pbb[:, :])

            for b in range(BPC):
                # load natural enc tiles
                et_nat = sba.tile([128, D2], F32, tag="etnat")
                nc.sync.dma_start(et_nat[:, :], enc["t"][b])
                en_nat = []
                for i in range(4):
                    t_ = sba.tile([128, D2], F32, tag=f"ennat{i}")
                    nc.sync.dma_start(t_[:, :], enc["n"][b, 128 * i : 128 * (i + 1), :])
                    en_nat.append(t_)
                # build encT via PE transposes
                eTt = sba.tile([128, 8 * 128], F32, tag="eTt")
                for j in range(8):
                    ptr = psa.tile([128, 128], F32, tag="patr")
                    nc.tensor.transpose(ptr[:, :], et_nat[:, 128 * j : 128 * (j + 1)],
                                        identity[:, :])
                    nc.vector.tensor_copy(eTt[:, 128 * j : 128 * (j + 1)], ptr[:, :])
                nc.sync.dma_start(encT["t"][b], eTt[:, :])
                eTn = []
                for j in range(8):
                    tj = sba.tile([128, SN], F32, tag=f"eTn{j}")
                    eTn.append(tj)
                for i in range(4):
                    for j in range(8):
                        ptr = psa.tile([128, 128], F32, tag="patr")
                        nc.tensor.transpose(
                            ptr[:, :], en_nat[i][:, 128 * j : 128 * (j + 1)],
                            identity[:, :])
                        nc.vector.tensor_copy(
                            eTn[j][:, 128 * i : 128 * (i + 1)], ptr[:, :])
                for j in range(8):
                    nc.sync.dma_start(
                        encT["n"][b, :, SN * j : SN * (j + 1)], eTn[j][:, :])

                # tnT [d2, n] = biatt_W @ encT_n + b
                tnT = []
                for j in range(8):
                    ptn = psa.tile([128, SN], F32, tag="ptn")
                    for k in range(8):
                        nc.tensor.matmul(
                            ptn[:, :],
                            lhsT=biatt_sb[k][:, 128 * j : 128 * (j + 1)],
                            rhs=eTn[k][:, :], start=(k == 0), stop=(k == 7))
                    tj = sba.tile([128, SN], F32, tag=f"tnT{j}")
                    nc.vector.tensor_scalar(
                        tj[:, :], ptn[:, :], bb_sb[:, j : j + 1], None, ALU.add)
                    tnT.append(tj)
                # ttT [d2, t]
                ttT = sba.tile([128, 8 * 128], F32, tag="ttT")
                for j in range(8):
                    ptt = psa.tile([128, 128], F32, tag="ptt")
                    for k in range(8):
                        nc.tensor.matmul(
                            ptt[:, :],
                            lhsT=biatt_sb[k][:, 128 * j : 128 * (j + 1)],
                            rhs=eTt[:, 128 * k : 128 * (k + 1)],
                            start=(k == 0), stop=(k == 7))
                    nc.vector.tensor_scalar(
                        ttT[:, 128 * j : 128 * (j + 1)], ptt[:, :],
                        bb_sb[:, j : j + 1], None, ALU.add)

                # scores_tn [t, n] + softmax + aT
                psc_tn = psa.tile([128, SN], F32, tag="psctn")
                for k in range(8):
                    nc.tensor.matmul(
                        psc_tn[:, :], lhsT=eTt[:, 128 * k : 128 * (k + 1)],
                        rhs=tnT[k][:, :], start=(k == 0), stop=(k == 7))
                a_tn = _softmax_free(nc, sba, psc_tn, SN, "tn")
                aTtn = sba.tile([128, SN], F32, tag="aTtn")  # [n, t] chunks
                for i in range(4):
                    ptr = psa.tile([128, 128], F32, tag="patr")
                    nc.tensor.transpose(ptr[:, :], a_tn[:, 128 * i : 128 * (i + 1)],
                                        identity[:, :])
                    nc.vector.tensor_copy(aTtn[:, 128 * i : 128 * (i + 1)], ptr[:, :])
                # r_newsT [d, t] = enc_n^T(nat lhsT) @ aT
                rn_sb = sba.tile([128, 8 * 128], F32, tag="rnsb")
                for j in range(8):
                    prn = psa.tile([128, 128], F32, tag="prn")
                    for i in range(4):
                        nc.tensor.matmul(
                            prn[:, :],
                            lhsT=en_nat[i][:, 128 * j : 128 * (j + 1)],
                            rhs=aTtn[:, 128 * i : 128 * (i + 1)],
                            start=(i == 0), stop=(i == 3))
                    nc.vector.tensor_copy(rn_sb[:, 128 * j : 128 * (j + 1)],
                                          prn[:, :])
                nc.sync.dma_start(rnT[b], rn_sb[:, :])

                # scores_nt [n, t] + softmax (4 chunks) + aT
                aTnt = sba.tile([128, SN], F32, tag="aTnt")  # [t, n]
                for i in range(4):
                    psc_nt = psa.tile([128, 128], F32, tag="pscnt")
                    for k in range(8):
                        nc.tensor.matmul(
                            psc_nt[:, :],
                            lhsT=eTn[k][:, 128 * i : 128 * (i + 1)],
                            rhs=ttT[:, 128 * k : 128 * (k + 1)],
                            start=(k == 0), stop=(k == 7))
                    a_i = _softmax_free(nc, sba, psc_nt, 128, "nt")
                    ptr = psa.tile([128, 128], F32, tag="patr")
                    nc.tensor.transpose(ptr[:, :], a_i[:, :], identity[:, :])
                    nc.vector.tensor_copy(aTnt[:, 128 * i : 128 * (i + 1)], ptr[:, :])
                # r_tweetsT [d, n] = enc_t(nat lhsT) @ aTnt
                rt_sb = sba.tile([128, 8 * SN], F32, tag="rtsb")
                for j in range(8):
                    prt = psa.tile([128, SN], F32, tag="prt")
                    nc.tensor.matmul(
                        prt[:, :], lhsT=et_nat[:, 128 * j : 128 * (j + 1)],
                        rhs=aTnt[:, :], start=True, stop=True)
                    nc.vector.tensor_copy(rt_sb[:, SN * j : SN * (j + 1)], prt[:, :])
                nc.sync.dma_start(rtT[b], rt_sb[:, :])
        tc.strict_bb_all_engine_barrier()

        # ================= phase 3B: v_t =====================================
        with tc.tile_pool(name="sbb", bufs=2) as sbb, \
             tc.tile_pool(name="psb", bufs=2, space="PSUM") as psb:
            wt_sb = []
            for k in range(16):
                wt_ = sbb.tile([128, D2], F32, tag=f"wt{k}", bufs=1)
                nc.sync.dma_start(wt_[:, :], wt_aug[128 * k : 128 * (k + 1), :])
                wt_sb.append(wt_)
            wt_bias = sbb.tile([1, D2], F32, tag="wtb", bufs=1)
            nc.sync.dma_start(wt_bias[:, :], wt_aug[2 * D2 : 2 * D2 + 1, :])
            for b in range(BPC):
                eTt = sbb.tile([128, 8 * 128], F32, tag="eTtb")
                nc.sync.dma_start(eTt[:, :], encT["t"][b])
                rn_sb = sbb.tile([128, 8 * 128], F32, tag="rnb")
                nc.sync.dma_start(rn_sb[:, :], rnT[b])
                pv = psb.tile([128, D2], F32, tag="pv")
                for n2 in range(2):
                    nsl = slice(512 * n2, 512 * (n2 + 1))
                    nc.tensor.matmul(pv[:, nsl], lhsT=ones_row[:, :],
                                     rhs=wt_bias[:, nsl], start=True, stop=False)
                    for k in range(16):
                        lhs = (eTt[:, 128 * k : 128 * (k + 1)] if k < 8
                               else rn_sb[:, 128 * (k - 8) : 128 * (k - 7)])
                        nc.tensor.matmul(pv[:, nsl], lhsT=lhs,
                                         rhs=wt_sb[k][:, nsl],
                                         start=False, stop=(k == 15))
                vt = sbb.tile([128, D2], F32, tag="vt")
                nc.scalar.activation(vt[:, :], pv[:, :], AF.Tanh)
                nc.sync.dma_start(enc_out[b, 0:ST, :], vt[:, :])
        tc.strict_bb_all_engine_barrier()

        # ================= phase 3C: v_n =====================================
        with tc.tile_pool(name="sbn", bufs=2) as sbn, \
             tc.tile_pool(name="psn", bufs=2, space="PSUM") as psn:
            wn_sb = []
            for k in range(16):
                wn_ = sbn.tile([128, D2], F32, tag=f"wn{k}", bufs=1)
                nc.sync.dma_start(wn_[:, :], wn_aug[128 * k : 128 * (k + 1), :])
                wn_sb.append(wn_)
            wn_bias = sbn.tile([1, D2], F32, tag="wnb", bufs=1)
            nc.sync.dma_start(wn_bias[:, :], wn_aug[2 * D2 : 2 * D2 + 1, :])
            for b in range(BPC):
                eTn = []
                for j in range(8):
                    tj = sbn.tile([128, SN], F32, tag=f"eTnc{j}")
                    nc.sync.dma_start(tj[:, :], encT["n"][b, :, SN * j : SN * (j + 1)])
                    eTn.append(tj)
                rt_sb = sbn.tile([128, 8 * SN], F32, tag="rtb")
                nc.sync.dma_start(rt_sb[:, :], rtT[b])
                for m in range(4):  # n chunks of 128
                    pv = psn.tile([128, D2], F32, tag="pvn")
                    for n2 in range(2):
                        nsl = slice(512 * n2, 512 * (n2 + 1))
                        nc.tensor.matmul(pv[:, nsl], lhsT=ones_row[:, :],
                                         rhs=wn_bias[:, nsl], start=True, stop=False)
                        for k in range(16):
                            lhs = (eTn[k][:, 128 * m : 128 * (m + 1)] if k < 8
                                   else rt_sb[:, SN * (k - 8) + 128 * m :
                                              SN * (k - 8) + 128 * (m + 1)])
                            nc.tensor.matmul(pv[:, nsl], lhsT=lhs,
                                             rhs=wn_sb[k][:, nsl],
                                             start=False, stop=(k == 15))
                    vn = sbn.tile([128, D2], F32, tag="vn")
                    nc.scalar.activation(vn[:, :], pv[:, :], AF.Tanh)
                    nc.sync.dma_start(
                        enc_out[b, ST + 128 * m : ST + 128 * (m + 1), :], vn[:, :])

    nc.compile()
    return nc


# ---------------------------------------------------------------- host side

_NC_CACHE = {}


def _get_program():
    if "nc" not in _NC_CACHE:
        t0 = time.time()
        _NC_CACHE["nc"] = build_program()
        print(f"[kernel] program build+compile: {time.time() - t0:.1f}s",
              file=sys.stderr)
    return _NC_CACHE["nc"]


def _prep_in_maps(inputs):
    f32 = lambda x: np.ascontiguousarray(np.asarray(x), dtype=np.float32)
    i32 = lambda x: np.ascontiguousarray(np.asarray(x), dtype=np.int32)

    tok = {"t": i32(inputs["input_tweets"]), "n": i32(inputs["input_news"])}
    for s in STREAMS:
        tok[s] = np.where(tok[s] > V, 3, tok[s]).astype(np.int32)
    lens = {"t": i32(inputs["lengths_tweets"]), "n": i32(inputs["lengths_news"])}

    emb = f32(inputs["emb_W"])
    shared = {"emb": emb, "biatt": f32(inputs["biatt_W"]).T.copy(),
              "biatt_b": f32(inputs["biatt_b"])}
    for s, pre in (("t", "gt"), ("n", "gn")):
        for d, dn in ((0, "f"), (1, "b")):
            Wih = f32(inputs[f"{pre}_Wih_{dn}"])
            Whh = f32(inputs[f"{pre}_Whh_{dn}"])
            bih = f32(inputs[f"{pre}_bih_{dn}"])
            bhh = f32(inputs[f"{pre}_bhh_{dn}"])
            bias = bih.copy()
            bias[: 2 * H] += bhh[: 2 * H]
            shared[f"wih_{s}{d}"] = np.ascontiguousarray(
                np.vstack([Wih.T, bias[None, :]]))
            shared[f"whh_{s}{d}"] = np.ascontiguousarray(Whh.T)
    bhhn = np.zeros((2, 2, H), np.float32)
    for si, (s, pre) in enumerate((("t", "gt"), ("n", "gn"))):
        for d, dn in ((0, "f"), (1, "b")):
            bhhn[si, d] = f32(inputs[f"{pre}_bhh_{dn}"])[2 * H :]
    shared["bhhn"] = bhhn
    shared["wt_aug"] = np.ascontiguousarray(
        np.vstack([f32(inputs["wt_W"]).T, f32(inputs["wt_b"])[None, :]]))
    shared["wn_aug"] = np.ascontiguousarray(
        np.vstack([f32(inputs["wn_W"]).T, f32(inputs["wn_b"])[None, :]]))
    shared["comb_aug"] = np.ascontiguousarray(
        np.vstack([f32(inputs["comb_W"]).T, f32(inputs["comb_b"])[None, :]]))

    in_maps = []
    for c in range(NCORES):
        bs = slice(c * BPC, (c + 1) * BPC)
        m = dict(shared)
        for s in STREAMS:
            T = TLEN[s]
            m[f"idx_{s}"] = np.ascontiguousarray(tok[s][bs])
            ln = lens[s][bs]
            t_ar = np.arange(T)
            mf = (t_ar[:, None] < ln[None, :]).astype(np.float32)      # [T, 8]
            mb = ((T - 1 - t_ar)[:, None] < ln[None, :]).astype(np.float32)
            marr = np.zeros((80, T), np.float32)
            marr[0:8] = mf.T
            marr[32:40] = mb.T
            marr[40:48] = 1.0 - mf.T
            marr[72:80] = 1.0 - mb.T
            m[f"masks_{s}"] = np.ascontiguousarray(marr)
        in_maps.append(m)
    return in_maps


def kernel(**inputs):
    nc = _get_program()
    in_maps = _prep_in_maps(inputs)
    res = run_bass_kernel_spmd(nc, in_maps, list(range(NCORES)))
    enc = np.concatenate([res.results[c]["enc_out"] for c in range(NCORES)], 0)
    hid = np.concatenate([res.results[c]["hid_out"] for c in range(NCORES)], 1)
    return enc.astype(np.float32), hid.astype(np.float32)


if __name__ == "__main__":
    inp = dict(np.load("/root/problem/inputs.npz"))
    t0 = time.time()
    out, hid = kernel(**inp)
    print("total wall:", time.time() - t0)
    exp = np.load("/root/problem/expected.npz")
    for name, got, ex in (("enc", out, exp["out"]), ("hid", hid, exp["hid"])):
        err = np.abs(got - ex).max()
        print(f"{name}: absmax_err={err:.3e} scale={np.abs(ex).max():.3f}")


# revision 12
# speedup vs baseline: 8.9166x; 5.4652x over previous
"""BiAttentionEncoder Trainium2 kernel (8 NeuronCores, data-parallel over batch).

Strategy (sharding_hint: data-parallel over batch):
  - 8 cores x 8 batch elements each; one SPMD program, per-core input shards.
  - Per core: embedding gather (indirect DMA) -> input projections (PE) ->
    bidirectional GRU recurrences (f+b stacked in PSUM partitions, freeze via
    z-gate mask trick) -> cross bilinear attention in d-major (transposed)
    layout -> tanh projections -> outputs. Host concatenates core outputs.

GRU per-step layout: PSUM tile [48, 512] rows = {f-r, b-r, f-z, b-z, f-n, b-n}
(8 rows each, batch in partitions); gates computed on [32,512]/[16,512] slices;
carry transposed each step via 4 PE transposes into the next step's lhsT.
Bias folding: bih + bhh for r/z folded into the phase-0 projection; bhh_n enters
the n-gate PSUM via a K=1 ones-row matmul (it must be scaled by r, so it cannot
be folded into xp).
"""

import os
import sys
import time

sys.path.insert(0, "/opt/trn_rl_repo")

import numpy as np

import concourse.bass as bass
import concourse.tile as tile
from concourse import bacc, mybir
from concourse.bass import IndirectOffsetOnAxis
from concourse.bass_utils import run_bass_kernel_spmd
from concourse.masks import make_identity

F32 = mybir.dt.float32
I32 = mybir.dt.int32
ALU = mybir.AluOpType
AF = mybir.ActivationFunctionType
AX = mybir.AxisListType

V, E, H = 30000, 300, 512
B, ST, SN = 64, 128, 512
NCORES = 8
BPC = B // NCORES  # 8 batch elements per core
D2 = 2 * H  # 1024
STREAMS = ("t", "n")
TLEN = {"t": ST, "n": SN}


# ---------------------------------------------------------------- device build


def _gather_and_xp(nc, tc, pools, stream, emb_ap, idx_ap, wih_tiles, xp_ap, identity):
    """Phase 0 for one stream: gather emb rows, transpose, project to xp.

    xp_ap: DRAM [T, 48, 512]; row layout g*16 + dir*8 + b.
    """
    T = TLEN[stream]
    sb, ps = pools["sb0"], pools["ps0"]
    for b in range(BPC):
        for tc0 in range(T // 128):
            t0 = tc0 * 128
            idx_sb = sb.tile([128, 1], I32, tag="idx")
            nc.sync.dma_start(idx_sb[:, :], idx_ap[b, t0 : t0 + 128].rearrange("(a b) -> a b", b=1))
            emb_sb = sb.tile([128, E], F32, tag="embg")
            nc.gpsimd.indirect_dma_start(
                out=emb_sb[:, :],
                out_offset=None,
                in_=emb_ap[:, :],
                in_offset=IndirectOffsetOnAxis(ap=idx_sb[:, :1], axis=0),
            )
            # transpose to [E(+1 ones row), 128] chunks: 128, 128, 44(+ones)
            embT = []
            for k, (e0, ke) in enumerate(((0, 128), (128, 128), (256, 44))):
                ptr = ps.tile([128, 128], F32, tag="p0tr")
                nc.tensor.transpose(
                    ptr[:ke, :], emb_sb[:, e0 : e0 + ke], identity[:, :]
                )
                kk = ke if k < 2 else ke + 1
                et = sb.tile([kk, 128], F32, tag=f"embT{k}")
                if k == 2:
                    nc.vector.memset(et[:, :], 1.0)  # row 44 stays 1 (bias row)
                nc.vector.tensor_copy(et[:ke, :], ptr[:ke, :])
                embT.append(et)
            for d in range(2):  # 0=f, 1=b
                pxp = ps.tile([128, 3 * 512], F32, tag="p0xp")
                for g in range(3):
                    for k in range(3):
                        kk = embT[k].shape[0]
                        nc.tensor.matmul(
                            pxp[:, 512 * g : 512 * (g + 1)],
                            lhsT=embT[k][:, :],
                            rhs=wih_tiles[(stream, d)][k][:kk, 512 * g : 512 * (g + 1)],
                            start=(k == 0),
                            stop=(k == 2),
                        )
                # write [128, 3, 512] -> xp rows (g*16 + d*8 + b), times t0..t0+128
                xps = sb.tile([128, 3 * 512], F32, tag="xpsb")
                nc.scalar.copy(xps[:, :], pxp[:, :])
                dst = xp_ap[t0 : t0 + 128, d * 8 + b :: 16, :]
                nc.sync.dma_start(dst, xps[:, :].rearrange("p (g d) -> p g d", g=3))


def _gru_step(nc, sb, ps, cx, s):
    """One step of one stream's bidirectional GRU. cx holds stream context."""
    st = cx["stream"]
    T = cx["T"]
    xp_ap = cx["xp"]
    whh = cx["whh"]
    t_f, t_b = s, T - 1 - s
    xpt = sb.tile([104, 512], F32, tag=f"xpt{st}", bufs=3, name=f"xpt{st}")
    xptn = sb.tile([40, 512], F32, tag=f"xptn{st}", bufs=3, name=f"xptn{st}")
    nc.sync.dma_start(xpt[0:8, :], xp_ap[t_f, 0:8, :])
    nc.sync.dma_start(xpt[32:40, :], xp_ap[t_b, 8:16, :])
    nc.sync.dma_start(xpt[64:72, :], xp_ap[t_f, 16:24, :])
    nc.sync.dma_start(xpt[96:104, :], xp_ap[t_b, 24:32, :])
    nc.sync.dma_start(xptn[0:8, :], xp_ap[t_f, 32:40, :])
    nc.sync.dma_start(xptn[32:40, :], xp_ap[t_b, 40:48, :])

    hT_prev = cx["hT"]
    ghrz = ps.tile([128, 512], F32, tag=f"ghrz{st}", bufs=1, name=f"ghrz{st}")
    ghn = ps.tile([40, 512], F32, tag=f"ghn{st}", bufs=1, name=f"ghn{st}")
    # r/z matmuls first so the rz-chain starts as early as possible
    for k in range(4):
        for d in range(2):
            lhs = hT_prev[:, 40 * k + 32 * d : 40 * k + 32 * d + 8]
            nc.tensor.matmul(ghrz[32 * d : 32 * d + 8, :], lhsT=lhs,
                             rhs=whh[d][k][:, 0:512],
                             start=(k == 0), stop=(k == 3))
            nc.tensor.matmul(ghrz[64 + 32 * d : 64 + 32 * d + 8, :], lhsT=lhs,
                             rhs=whh[d][k][:, 512:1024],
                             start=(k == 0), stop=(k == 3),
                             tile_position=(0, 64 + 32 * d))
    # n-gate: bhh_n preload via ones-row matmul (start=True clears psum)
    for d in range(2):
        nc.tensor.matmul(ghn[32 * d : 32 * d + 8, :], lhsT=cx["ones"][:, :8],
                         rhs=cx["bhhn"][d][:, :], start=True, stop=False)
    for k in range(4):
        for d in range(2):
            lhs = hT_prev[:, 40 * k + 32 * d : 40 * k + 32 * d + 8]
            nc.tensor.matmul(ghn[32 * d : 32 * d + 8, :], lhsT=lhs,
                             rhs=whh[d][k][:, 1024:1536],
                             start=False, stop=(k == 3))

    masks_m, masks_c = cx["masks_m"], cx["masks_c"]
    rz = sb.tile([104, 512], F32, tag=f"rz{st}", bufs=1, name=f"rz{st}")
    nc.vector.tensor_tensor(rz[:, :], ghrz[0:104, :], xpt[:, :], op=ALU.add)
    rzs = sb.tile([104, 512], F32, tag=f"rzs{st}", bufs=1, name=f"rzs{st}")
    nc.scalar.activation(rzs[:, :], rz[:, :], AF.Sigmoid)
    zm = sb.tile([40, 512], F32, tag=f"zm{st}", bufs=1, name=f"zm{st}")
    nc.vector.tensor_scalar(
        zm[:, :], rzs[64:104, :], masks_m[:, s : s + 1],
        masks_c[:, s : s + 1], ALU.mult, ALU.add,
    )
    nh = sb.tile([40, 512], F32, tag=f"nh{st}", bufs=1, name=f"nh{st}")
    nc.vector.tensor_tensor(nh[:, :], ghn[0:40, :], rzs[0:40, :], op=ALU.mult)
    npre = sb.tile([40, 512], F32, tag=f"npre{st}", bufs=1, name=f"npre{st}")
    nc.vector.tensor_tensor(npre[:, :], nh[:, :], xptn[:, :], op=ALU.add)
    ngate = sb.tile([40, 512], F32, tag=f"ngate{st}", bufs=1, name=f"ngate{st}")
    nc.scalar.activation(ngate[:, :], npre[:, :], AF.Tanh)
    h_prev = cx["h"]
    dd = sb.tile([40, 512], F32, tag=f"dd{st}", bufs=1, name=f"dd{st}")
    nc.vector.tensor_tensor(dd[:, :], h_prev[:, :], ngate[:, :], op=ALU.subtract)
    ee = sb.tile([40, 512], F32, tag=f"ee{st}", bufs=1, name=f"ee{st}")
    nc.vector.tensor_tensor(ee[:, :], zm[:, :], dd[:, :], op=ALU.mult)
    h_new = sb.tile([40, H], F32, tag=f"h{st}", bufs=2, name=f"h{st}")
    nc.vector.tensor_tensor(h_new[:, :], ngate[:, :], ee[:, :], op=ALU.add)
    wm = sb.tile([40, 512], F32, tag=f"wm{st}", bufs=1, name=f"wm{st}")
    nc.gpsimd.tensor_scalar(
        wm[:, :], h_new[:, :], masks_m[:, s : s + 1], None, ALU.mult
    )
    enc_ap = cx["enc"]
    nc.sync.dma_start(enc_ap[0:BPC, t_f, 0:H], wm[0:8, :])
    nc.sync.dma_start(enc_ap[0:BPC, t_b, H:D2], wm[32:40, :])

    trp = ps.tile([128, 160], F32, tag=f"trp{st}", bufs=1, name=f"trp{st}")
    for j in range(4):
        nc.tensor.transpose(
            trp[:, 40 * j : 40 * (j + 1)],
            h_new[:, 128 * j : 128 * (j + 1)],
            cx["identity"][0:40, 0:40],
        )
    hT_new = sb.tile([128, 160], F32, tag=f"hT{st}", bufs=2, name=f"hT{st}")
    nc.vector.tensor_copy(hT_new[:, :], trp[:, :])
    cx["h"], cx["hT"] = h_new, hT_new


def _softmax_free(nc, sb, psc, n_free, tag):
    """softmax over free dim of PSUM tile psc [P, n_free] -> sbuf tile."""
    mxn = sb.tile([128, 1], F32, tag=f"mx{tag}")
    nc.vector.tensor_reduce(mxn[:, :], psc[:, :], axis=AX.X, op=ALU.max, negate=True)
    ssum = sb.tile([128, 1], F32, tag=f"ss{tag}")
    ae = sb.tile([128, n_free], F32, tag=f"ae{tag}")
    nc.scalar.activation(ae[:, :], psc[:, :], AF.Exp, bias=mxn[:, :],
                         accum_out=ssum[:, :])
    rinv = sb.tile([128, 1], F32, tag=f"ri{tag}")
    nc.vector.reciprocal(rinv[:, :], ssum[:, :])
    a = sb.tile([128, n_free], F32, tag=f"a{tag}")
    nc.vector.tensor_scalar(a[:, :], ae[:, :], rinv[:, :], None, ALU.mult)
    return a


def build_program():
    nc = bacc.Bacc("TRN2", target_bir_lowering=False, debug=False,
                   num_devices=NCORES)

    # ---- I/O ----
    emb = nc.dram_tensor("emb", [V, E], F32, kind="ExternalInput").ap()
    idx = {s: nc.dram_tensor(f"idx_{s}", [BPC, TLEN[s]], I32,
                             kind="ExternalInput").ap() for s in STREAMS}
    masks = {s: nc.dram_tensor(f"masks_{s}", [80, TLEN[s]], F32,
                               kind="ExternalInput").ap() for s in STREAMS}
    wih = {}
    whh = {}
    for s in STREAMS:
        for d in range(2):
            wih[(s, d)] = nc.dram_tensor(f"wih_{s}{d}", [301, 3 * H], F32,
                                         kind="ExternalInput").ap()
            whh[(s, d)] = nc.dram_tensor(f"whh_{s}{d}", [H, 3 * H], F32,
                                         kind="ExternalInput").ap()
    bhhn = nc.dram_tensor("bhhn", [2, 2, H], F32, kind="ExternalInput").ap()
    biatt = nc.dram_tensor("biatt", [D2, D2], F32, kind="ExternalInput").ap()
    biatt_b = nc.dram_tensor("biatt_b", [D2], F32, kind="ExternalInput").ap()
    wt_aug = nc.dram_tensor("wt_aug", [2 * D2 + 1, D2], F32,
                            kind="ExternalInput").ap()
    wn_aug = nc.dram_tensor("wn_aug", [2 * D2 + 1, D2], F32,
                            kind="ExternalInput").ap()
    comb_aug = nc.dram_tensor("comb_aug", [D2 + 1, H], F32,
                              kind="ExternalInput").ap()

    enc_out = nc.dram_tensor("enc_out", [BPC, ST + SN, D2], F32,
                             kind="ExternalOutput").ap()
    hid_out = nc.dram_tensor("hid_out", [2, BPC, H], F32,
                             kind="ExternalOutput").ap()

    # ---- internal DRAM ----
    xp = {s: nc.dram_tensor(f"xp_{s}", [TLEN[s], 48, 512], F32).ap()
          for s in STREAMS}
    enc = {s: nc.dram_tensor(f"enc_{s}", [BPC, TLEN[s], D2], F32).ap()
           for s in STREAMS}
    hidT = nc.dram_tensor("hidT", [2, 128, 160], F32).ap()
    encT = {s: nc.dram_tensor(f"encT_{s}", [BPC, 128, 8 * TLEN[s]], F32).ap()
            for s in STREAMS}  # cols = (dchunk j, t)
    rnT = nc.dram_tensor("rnT", [BPC, 128, 8 * ST], F32).ap()   # r_news^T
    rtT = nc.dram_tensor("rtT", [BPC, 128, 8 * SN], F32).ap()   # r_tweets^T

    with tile.TileContext(nc) as tc, tc.tile_pool(name="const", bufs=1) as const:
        identity = const.tile([128, 128], F32)
        make_identity(nc, identity[:, :])
        ones_row = const.tile([1, 128], F32)
        nc.vector.memset(ones_row[:, :], 1.0)

        # ================= phase 0: gather + input projections ==============
        with tc.tile_pool(name="sb0", bufs=2) as sb0, \
             tc.tile_pool(name="ps0", bufs=2, space="PSUM") as ps0:
            pools = {"sb0": sb0, "ps0": ps0}
            wih_tiles = {}
            for s in STREAMS:
                for d in range(2):
                    tiles = []
                    for k, (e0, ke) in enumerate(((0, 128), (128, 128), (256, 45))):
                        wt_ = sb0.tile([ke, 3 * H], F32, tag=f"wih{s}{d}{k}",
                                       bufs=1)
                        nc.sync.dma_start(wt_[:, :], wih[(s, d)][e0 : e0 + ke, :])
                        tiles.append(wt_)
                    wih_tiles[(s, d)] = tiles
            for s in STREAMS:
                _gather_and_xp(nc, tc, pools, s, emb, idx[s], wih_tiles, xp[s],
                               identity)
        tc.strict_bb_all_engine_barrier()

        # ================= phases 1-2: GRU recurrences ======================
        # news and tweets recurrences are independent serial chains; emit
        # them interleaved so the (latency-bound) chains share engines.
        with tc.tile_pool(name="sb1", bufs=2) as sb1, \
             tc.tile_pool(name="ps1", bufs=1, space="PSUM") as ps1:
            ctxs = {}
            for st in ("n", "t"):
                whh_tiles = []
                for d in range(2):
                    tiles = []
                    for k in range(4):
                        wt_ = sb1.tile([128, 3 * H], F32, tag=f"whh{st}{d}{k}",
                                       bufs=1, name=f"whh{st}{d}{k}")
                        nc.sync.dma_start(
                            wt_[:, :], whh[(st, d)][128 * k : 128 * (k + 1), :]
                        )
                        tiles.append(wt_)
                    whh_tiles.append(tiles)
                si_ = 0 if st == "t" else 1
                bf = sb1.tile([1, H], F32, tag=f"bhhnf{st}", bufs=1,
                              name=f"bhhnf{st}")
                nc.sync.dma_start(bf[:, :],
                                  bhhn[si_, 0].rearrange("(a b) -> a b", a=1))
                bb_ = sb1.tile([1, H], F32, tag=f"bhhnb{st}", bufs=1,
                               name=f"bhhnb{st}")
                nc.sync.dma_start(bb_[:, :],
                                  bhhn[si_, 1].rearrange("(a b) -> a b", a=1))
                masks_m = sb1.tile([40, TLEN[st]], F32, tag=f"masksm{st}",
                                   bufs=1, name=f"masksm{st}")
                nc.sync.dma_start(masks_m[:, :], masks[st][0:40, :])
                masks_c = sb1.tile([40, TLEN[st]], F32, tag=f"masksc{st}",
                                   bufs=1, name=f"masksc{st}")
                nc.sync.dma_start(masks_c[:, :], masks[st][40:80, :])
                h0 = sb1.tile([40, H], F32, tag=f"h{st}", bufs=2, name=f"h{st}")
                nc.vector.memset(h0[:, :], 0.0)
                hT0 = sb1.tile([128, 160], F32, tag=f"hT{st}", bufs=2,
                               name=f"hT{st}")
                nc.vector.memset(hT0[:, :], 0.0)
                ctxs[st] = {
                    "stream": st, "T": TLEN[st], "xp": xp[st],
                    "whh": whh_tiles, "bhhn": (bf, bb_),
                    "masks_m": masks_m, "masks_c": masks_c,
                    "enc": enc[st], "h": h0, "hT": hT0,
                    "ones": ones_row, "identity": identity,
                }
            n_steps = {"n": TLEN["n"], "t": TLEN["t"]}
            if os.environ.get("KERNEL_SKIP_GRU"):
                n_steps = {"n": 4, "t": 4}
            for s_ in range(n_steps["n"]):
                _gru_step(nc, sb1, ps1, ctxs["n"], s_)
                if s_ < n_steps["t"]:
                    _gru_step(nc, sb1, ps1, ctxs["t"], s_)
            nc.sync.dma_start(hidT[1], ctxs["n"]["hT"][:, :])
            nc.sync.dma_start(hidT[0], ctxs["t"]["hT"][:, :])
        tc.strict_bb_all_engine_barrier()

        # ================= hidden: comb projection ==========================
        with tc.tile_pool(name="sbc", bufs=1) as sbc, \
             tc.tile_pool(name="psc", bufs=1, space="PSUM") as psc:
            comb_sb = []
            for k in range(8):
                ct = sbc.tile([128, H], F32, tag=f"comb{k}")
                nc.sync.dma_start(ct[:, :], comb_aug[128 * k : 128 * (k + 1), :])
                comb_sb.append(ct)
            comb_bias = sbc.tile([1, H], F32, tag="combb")
            nc.sync.dma_start(comb_bias[:, :], comb_aug[D2 : D2 + 1, :])
            hidT_sb = {}
            for si in range(2):
                for j in range(4):
                    ht = sbc.tile([128, 16], F32, tag=f"hidT{si}{j}")
                    src = hidT[si].rearrange("p (j g r) -> p j g r", j=4, g=5)
                    nc.sync.dma_start(ht[:, :], src[:, j, 0::4, :])
                    hidT_sb[(si, j)] = ht
            ph = psc.tile([16, H], F32, tag="phid")
            nc.tensor.matmul(ph[:, :], lhsT=ones_row[:, :16],
                             rhs=comb_bias[:, :], start=True, stop=False)
            for si in range(2):  # 0=tweets dims 0-511, 1=news dims 512-1023
                for j in range(4):
                    nc.tensor.matmul(
                        ph[:, :],
                        lhsT=hidT_sb[(si, j)][:, :],
                        rhs=comb_sb[4 * si + j][:, :],
                        start=False,
                        stop=(si == 1 and j == 3),
                    )
            hid_sb = sbc.tile([16, H], F32, tag="hid")
            nc.vector.tensor_copy(hid_sb[:, :], ph[:, :])
            nc.sync.dma_start(hid_out.rearrange("a b d -> (a b) d"), hid_sb[:, :])
        tc.strict_bb_all_engine_barrier()

        # ================= phase 3A: biatt + scores + r ====================
        with tc.tile_pool(name="sba", bufs=1) as sba, \
             tc.tile_pool(name="psa", bufs=1, space="PSUM") as psa:
            biatt_sb = []
            for k in range(8):
                bt = sba.tile([128, D2], F32, tag=f"biatt{k}", bufs=1)
                nc.sync.dma_start(bt[:, :], biatt[128 * k : 128 * (k + 1), :])
                biatt_sb.append(bt)
            bb_sb = sba.tile([128, 8], F32, tag="biattb", bufs=1)
            # biatt_b as [128,1] per chunk: load as [8,128] rows -> transpose
            bbT_tmp = sba.tile([8, 128], F32, tag="bbtmp", bufs=1)
            nc.sync.dma_start(bbT_tmp[:, :], biatt_b.rearrange("(a b) -> a b", a=8))
            pbb = psa.tile([128, 8], F32, tag="pbb", bufs=1)
            nc.tensor.transpose(pbb[:, :], bbT_tmp[:, :], identity[0:8, 0:8])
            nc.vector.tensor_copy(bb_sb[:, :], pbb[:, :])

            for b in range(BPC):
                # load natural enc tiles
                et_nat = sba.tile([128, D2], F32, tag="etnat")
                nc.sync.dma_start(et_nat[:, :], enc["t"][b])
                en_nat = []
                for i in range(4):
                    t_ = sba.tile([128, D2], F32, tag=f"ennat{i}")
                    nc.sync.dma_start(t_[:, :], enc["n"][b, 128 * i : 128 * (i + 1), :])
                    en_nat.append(t_)
                # build encT via PE transposes
                eTt = sba.tile([128, 8 * 128], F32, tag="eTt")
                for j in range(8):
                    ptr = psa.tile([128, 128], F32, tag="patr")
                    nc.tensor.transpose(ptr[:, :], et_nat[:, 128 * j : 128 * (j + 1)],
                                        identity[:, :])
                    nc.vector.tensor_copy(eTt[:, 128 * j : 128 * (j + 1)], ptr[:, :])
                nc.sync.dma_start(encT["t"][b], eTt[:, :])
                eTn = []
                for j in range(8):
                    tj = sba.tile([128, SN], F32, tag=f"eTn{j}")
                    eTn.append(tj)
                for i in range(4):
                    for j in range(8):
                        ptr = psa.tile([128, 128], F32, tag="patr")
                        nc.tensor.transpose(
                            ptr[:, :], en_nat[i][:, 128 * j : 128 * (j + 1)],
                            identity[:, :])
                        nc.vector.tensor_copy(
                            eTn[j][:, 128 * i : 128 * (i + 1)], ptr[:, :])
                for j in range(8):
                    nc.sync.dma_start(
                        encT["n"][b, :, SN * j : SN * (j + 1)], eTn[j][:, :])

                # tnT [d2, n] = biatt_W @ encT_n + b
                tnT = []
                for j in range(8):
                    ptn = psa.tile([128, SN], F32, tag="ptn")
                    for k in range(8):
                        nc.tensor.matmul(
                            ptn[:, :],
                            lhsT=biatt_sb[k][:, 128 * j : 128 * (j + 1)],
                            rhs=eTn[k][:, :], start=(k == 0), stop=(k == 7))
                    tj = sba.tile([128, SN], F32, tag=f"tnT{j}")
                    nc.vector.tensor_scalar(
                        tj[:, :], ptn[:, :], bb_sb[:, j : j + 1], None, ALU.add)
                    tnT.append(tj)
                # ttT [d2, t]
                ttT = sba.tile([128, 8 * 128], F32, tag="ttT")
                for j in range(8):
                    ptt = psa.tile([128, 128], F32, tag="ptt")
                    for k in range(8):
                        nc.tensor.matmul(
                            ptt[:, :],
                            lhsT=biatt_sb[k][:, 128 * j : 128 * (j + 1)],
                            rhs=eTt[:, 128 * k : 128 * (k + 1)],
                            start=(k == 0), stop=(k == 7))
                    nc.vector.tensor_scalar(
                        ttT[:, 128 * j : 128 * (j + 1)], ptt[:, :],
                        bb_sb[:, j : j + 1], None, ALU.add)

                # scores_tn [t, n] + softmax + aT
                psc_tn = psa.tile([128, SN], F32, tag="psctn")
                for k in range(8):
                    nc.tensor.matmul(
                        psc_tn[:, :], lhsT=eTt[:, 128 * k : 128 * (k + 1)],
                        rhs=tnT[k][:, :], start=(k == 0), stop=(k == 7))
                a_tn = _softmax_free(nc, sba, psc_tn, SN, "tn")
                aTtn = sba.tile([128, SN], F32, tag="aTtn")  # [n, t] chunks
                for i in range(4):
                    ptr = psa.tile([128, 128], F32, tag="patr")
                    nc.tensor.transpose(ptr[:, :], a_tn[:, 128 * i : 128 * (i + 1)],
                                        identity[:, :])
                    nc.vector.tensor_copy(aTtn[:, 128 * i : 128 * (i + 1)], ptr[:, :])
                # r_newsT [d, t] = enc_n^T(nat lhsT) @ aT
                rn_sb = sba.tile([128, 8 * 128], F32, tag="rnsb")
                for j in range(8):
                    prn = psa.tile([128, 128], F32, tag="prn")
                    for i in range(4):
                        nc.tensor.matmul(
                            prn[:, :],
                            lhsT=en_nat[i][:, 128 * j : 128 * (j + 1)],
                            rhs=aTtn[:, 128 * i : 128 * (i + 1)],
                            start=(i == 0), stop=(i == 3))
                    nc.vector.tensor_copy(rn_sb[:, 128 * j : 128 * (j + 1)],
                                          prn[:, :])
                nc.sync.dma_start(rnT[b], rn_sb[:, :])

                # scores_nt [n, t] + softmax (4 chunks) + aT
                aTnt = sba.tile([128, SN], F32, tag="aTnt")  # [t, n]
                for i in range(4):
                    psc_nt = psa.tile([128, 128], F32, tag="pscnt")
                    for k in range(8):
                        nc.tensor.matmul(
                            psc_nt[:, :],
                            lhsT=eTn[k][:, 128 * i : 128 * (i + 1)],
                            rhs=ttT[:, 128 * k : 128 * (k + 1)],
                            start=(k == 0), stop=(k == 7))
                    a_i = _softmax_free(nc, sba, psc_nt, 128, "nt")
                    ptr = psa.tile([128, 128], F32, tag="patr")
                    nc.tensor.transpose(ptr[:, :], a_i[:, :], identity[:, :])
                    nc.vector.tensor_copy(aTnt[:, 128 * i : 128 * (i + 1)], ptr[:, :])
                # r_tweetsT [d, n] = enc_t(nat lhsT) @ aTnt
                rt_sb = sba.tile([128, 8 * SN], F32, tag="rtsb")
                for j in range(8):
                    prt = psa.tile([128, SN], F32, tag="prt")
                    nc.tensor.matmul(
                        prt[:, :], lhsT=et_nat[:, 128 * j : 128 * (j + 1)],
                        rhs=aTnt[:, :], start=True, stop=True)
                    nc.vector.tensor_copy(rt_sb[:, SN * j : SN * (j + 1)], prt[:, :])
                nc.sync.dma_start(rtT[b], rt_sb[:, :])
        tc.strict_bb_all_engine_barrier()

        # ================= phase 3B: v_t =====================================
        with tc.tile_pool(name="sbb", bufs=2) as sbb, \
             tc.tile_pool(name="psb", bufs=2, space="PSUM") as psb:
            wt_sb = []
            for k in range(16):
                wt_ = sbb.tile([128, D2], F32, tag=f"wt{k}", bufs=1)
                nc.sync.dma_start(wt_[:, :], wt_aug[128 * k : 128 * (k + 1), :])
                wt_sb.append(wt_)
            wt_bias = sbb.tile([1, D2], F32, tag="wtb", bufs=1)
            nc.sync.dma_start(wt_bias[:, :], wt_aug[2 * D2 : 2 * D2 + 1, :])
            for b in range(BPC):
                eTt = sbb.tile([128, 8 * 128], F32, tag="eTtb")
                nc.sync.dma_start(eTt[:, :], encT["t"][b])
                rn_sb = sbb.tile([128, 8 * 128], F32, tag="rnb")
                nc.sync.dma_start(rn_sb[:, :], rnT[b])
                pv = psb.tile([128, D2], F32, tag="pv")
                for n2 in range(2):
                    nsl = slice(512 * n2, 512 * (n2 + 1))
                    nc.tensor.matmul(pv[:, nsl], lhsT=ones_row[:, :],
                                     rhs=wt_bias[:, nsl], start=True, stop=False)
                    for k in range(16):
                        lhs = (eTt[:, 128 * k : 128 * (k + 1)] if k < 8
                               else rn_sb[:, 128 * (k - 8) : 128 * (k - 7)])
                        nc.tensor.matmul(pv[:, nsl], lhsT=lhs,
                                         rhs=wt_sb[k][:, nsl],
                                         start=False, stop=(k == 15))
                vt = sbb.tile([128, D2], F32, tag="vt")
                nc.scalar.activation(vt[:, :], pv[:, :], AF.Tanh)
                nc.sync.dma_start(enc_out[b, 0:ST, :], vt[:, :])
        tc.strict_bb_all_engine_barrier()

        # ================= phase 3C: v_n =====================================
        with tc.tile_pool(name="sbn", bufs=2) as sbn, \
             tc.tile_pool(name="psn", bufs=2, space="PSUM") as psn:
            wn_sb = []
            for k in range(16):
                wn_ = sbn.tile([128, D2], F32, tag=f"wn{k}", bufs=1)
                nc.sync.dma_start(wn_[:, :], wn_aug[128 * k : 128 * (k + 1), :])
                wn_sb.append(wn_)
            wn_bias = sbn.tile([1, D2], F32, tag="wnb", bufs=1)
            nc.sync.dma_start(wn_bias[:, :], wn_aug[2 * D2 : 2 * D2 + 1, :])
            for b in range(BPC):
                eTn = []
                for j in range(8):
                    tj = sbn.tile([128, SN], F32, tag=f"eTnc{j}")
                    nc.sync.dma_start(tj[:, :], encT["n"][b, :, SN * j : SN * (j + 1)])
                    eTn.append(tj)
                rt_sb = sbn.tile([128, 8 * SN], F32, tag="rtb")
                nc.sync.dma_start(rt_sb[:, :], rtT[b])
                for m in range(4):  # n chunks of 128
                    pv = psn.tile([128, D2], F32, tag="pvn")
                    for n2 in range(2):
                        nsl = slice(512 * n2, 512 * (n2 + 1))
                        nc.tensor.matmul(pv[:, nsl], lhsT=ones_row[:, :],
                                         rhs=wn_bias[:, nsl], start=True, stop=False)
                        for k in range(16):
                            lhs = (eTn[k][:, 128 * m : 128 * (m + 1)] if k < 8
                                   else rt_sb[:, SN * (k - 8) + 128 * m :
                                              SN * (k - 8) + 128 * (m + 1)])
                            nc.tensor.matmul(pv[:, nsl], lhsT=lhs,
                                             rhs=wn_sb[k][:, nsl],
                                             start=False, stop=(k == 15))
                    vn = sbn.tile([128, D2], F32, tag="vn")
                    nc.scalar.activation(vn[:, :], pv[:, :], AF.Tanh)
                    nc.sync.dma_start(
                        enc_out[b, ST + 128 * m : ST + 128 * (m + 1), :], vn[:, :])

    nc.compile()
    return nc


# ---------------------------------------------------------------- host side

_NC_CACHE = {}


def _get_program():
    if "nc" not in _NC_CACHE:
        t0 = time.time()
        _NC_CACHE["nc"] = build_program()
        print(f"[kernel] program build+compile: {time.time() - t0:.1f}s",
              file=sys.stderr)
    return _NC_CACHE["nc"]


def _prep_in_maps(inputs):
    f32 = lambda x: np.ascontiguousarray(np.asarray(x), dtype=np.float32)
    i32 = lambda x: np.ascontiguousarray(np.asarray(x), dtype=np.int32)

    tok = {"t": i32(inputs["input_tweets"]), "n": i32(inputs["input_news"])}
    for s in STREAMS:
        tok[s] = np.where(tok[s] > V, 3, tok[s]).astype(np.int32)
    lens = {"t": i32(inputs["lengths_tweets"]), "n": i32(inputs["lengths_news"])}

    emb = f32(inputs["emb_W"])
    shared = {"emb": emb, "biatt": f32(inputs["biatt_W"]).T.copy(),
              "biatt_b": f32(inputs["biatt_b"])}
    for s, pre in (("t", "gt"), ("n", "gn")):
        for d, dn in ((0, "f"), (1, "b")):
            Wih = f32(inputs[f"{pre}_Wih_{dn}"])
            Whh = f32(inputs[f"{pre}_Whh_{dn}"])
            bih = f32(inputs[f"{pre}_bih_{dn}"])
            bhh = f32(inputs[f"{pre}_bhh_{dn}"])
            bias = bih.copy()
            bias[: 2 * H] += bhh[: 2 * H]
            shared[f"wih_{s}{d}"] = np.ascontiguousarray(
                np.vstack([Wih.T, bias[None, :]]))
            shared[f"whh_{s}{d}"] = np.ascontiguousarray(Whh.T)
    bhhn = np.zeros((2, 2, H), np.float32)
    for si, (s, pre) in enumerate((("t", "gt"), ("n", "gn"))):
        for d, dn in ((0, "f"), (1, "b")):
            bhhn[si, d] = f32(inputs[f"{pre}_bhh_{dn}"])[2 * H :]
    shared["bhhn"] = bhhn
    shared["wt_aug"] = np.ascontiguousarray(
        np.vstack([f32(inputs["wt_W"]).T, f32(inputs["wt_b"])[None, :]]))
    shared["wn_aug"] = np.ascontiguousarray(
        np.vstack([f32(inputs["wn_W"]).T, f32(inputs["wn_b"])[None, :]]))
    shared["comb_aug"] = np.ascontiguousarray(
        np.vstack([f32(inputs["comb_W"]).T, f32(inputs["comb_b"])[None, :]]))

    in_maps = []
    for c in range(NCORES):
        bs = slice(c * BPC, (c + 1) * BPC)
        m = dict(shared)
        for s in STREAMS:
            T = TLEN[s]
            m[f"idx_{s}"] = np.ascontiguousarray(tok[s][bs])
            ln = lens[s][bs]
            t_ar = np.arange(T)
            mf = (t_ar[:, None] < ln[None, :]).astype(np.float32)      # [T, 8]
            mb = ((T - 1 - t_ar)[:, None] < ln[None, :]).astype(np.float32)
            marr = np.zeros((80, T), np.float32)
            marr[0:8] = mf.T
            marr[32:40] = mb.T
            marr[40:48] = 1.0 - mf.T
            marr[72:80] = 1.0 - mb.T
            m[f"masks_{s}"] = np.ascontiguousarray(marr)
        in_maps.append(m)
    return in_maps


def kernel(**inputs):
    nc = _get_program()
    in_maps = _prep_in_maps(inputs)
    res = run_bass_kernel_spmd(nc, in_maps, list(range(NCORES)))
    enc = np.concatenate([res.results[c]["enc_out"] for c in range(NCORES)], 0)
    hid = np.concatenate([res.results[c]["hid_out"] for c in range(NCORES)], 1)
    return enc.astype(np.float32), hid.astype(np.float32)


if __name__ == "__main__":
    inp = dict(np.load("/root/problem/inputs.npz"))
    t0 = time.time()
    out, hid = kernel(**inp)
    print("total wall:", time.time() - t0)
    exp = np.load("/root/problem/expected.npz")
    for name, got, ex in (("enc", out, exp["out"]), ("hid", hid, exp["hid"])):
        err = np.abs(got - ex).max()
        print(f"{name}: absmax_err={err:.3e} scale={np.abs(ex).max():.3f}")
